# revision 1
# baseline (speedup 1.0000x reference)
"""AxialSelfAttention Trainium2 Bass kernel.

Reference computation (per batch b):
    xs  = x[b] reshaped [N=2048, E=512]
    qkv = xs @ W + bias                      # [N, 3E]
    q, k, v = split(qkv)
    row:  P = softmax(q @ k.T / sqrt(E));  out_row = P @ v
    col:  A = softmax(q.T @ k / sqrt(E));  out_col = v @ A.T
    out = out_row + out_col                  # [N, E]

Sharding: data-parallel over batch B=32 across 8 cores (4 batches/core).

Shipped variant: 51 = the full St-direct algorithm on race-free queues:
row attention computed transposed (St = K Q^T per 512-token j-group) so
exp() lands directly in P^T layout (no P transposes, no DVE copies on
that path); Qt/Kt/Vt/AcolT via bf16 DMA-XBAR transposes ALL on the
scalar queue; x/y DMA on gpsimd; row sums via a ones-column matmul
reusing the PV stationary; 1/rowsum applied at the DVE merge.
Device-verified BIT-DETERMINISTIC at rel err 1.2320e-3 across 5
single-shot trials. Model 993us/core (v13: 1022, but with PE busy only
736us vs v13's 900us, half the DVE work, 17% fewer instructions);
MEASURED real marginal 1.14ms/rep (reps-delta, floor 66.4ms at
reps=33) vs v13's 1.93ms (-41%) and v26's 1.22ms.
(v26 = v13+qkt_dma, also bit-stable, measured 1.22ms/rep, is the
fallback if anything regresses.)

Real-device-time model, fitted on the two reps-delta measurements
(v26: 1.22ms @ 1084us-model/14033 insts; v30: ~0.95ms @ 850us/11913):
  real_per_rep ~= model_ns + 9.7ns * instruction_count
(validated: predicted 1.107ms for v51, measured 1.14ms — within 3%)
Under it the remaining screened candidates (52: flat; 54: -0.6% model;
56: -1.9% model but +973 insts) are all within noise of v51 (~1.10ms).
Real further gains need PE-work cuts (blocked: fp8 fails accuracy) or
the ~850us spread-queue schedule (blocked: SP-queue XBAR corruption).

The v50/v51 'deterministic 0.866' bug was NOT a framework issue: a
scripted edit had de-indented the y-store out of the jj-loop so only
j=3,7,11,15 were stored. Fixed; v51 then passed 5/5 bit-identical.

ROOT CAUSE of the long-chased corruption: the spread_t queue layout.
v26 (no spread) is bit-stable; v27 (= v26 + spread_t: Qt-T on the SP
queue, Vt-T alternating, x/y DMA on gpsimd) corrupts nondeterministically
up to 5e-2. Every corrupt variant (27-48) had spread_t; both clean
lines (13, 26) do not. Suspected mechanism: XBAR-transpose issues on the
SP queue colliding with the tile framework's semaphore housekeeping
(BassTileRelease runs on SP), or the gpsimd SWDGE x/y path.

st_direct follow-ups (v50/v51): moving the Qt/Vt transposes to the
scalar queue inside st_direct gives a DETERMINISTIC rel err 0.866 —
1536/2048 output rows all-zero (12 of 16 j-chunks never stored), same
value on device and in CoreSim (v51 restored x/y to gpsimd and it did
not change). Since it reproduces in CoreSim it is debuggable offline:
run CoreSim nb=1 on v50, find which j-chunks are unwritten and walk the
B2 store chain back (likely a framework scheduling/dep issue when all
52 XBAR transposes + 68 exps share the ACT queue). Fixing that unlocks
~968us-model st_direct on race-free queues -> est. ~1.0-1.1ms/rep.

Variant 30+ ("St-direct", see FEAT below) is the faster research line:
row attention computed transposed (St = K Q^T per 512-token j-group) so
exp() lands directly in P^T layout, Qt/Kt via DMA-XBAR transposes, row
sums via a ones-column matmul reusing the PV stationary. It models at
850us/core vs 13's 1022us and measured ~2x lower marginal device time
per rep (reps-delta: ~0.95ms vs 1.93ms), BUT it shows a nondeterministic
whole-chunk corruption on hardware (batch 0 always clean, later batches
intermittently 0.004-0.018 global rel err for the v30 family, 0.06-0.13
for the v32 family; CoreSim and TimelineSim both clean => HW-only).

Race-hunt results (all on device, single-shot kernel() trials):
  - NOT the ptg single-buffer, Vn/Vt dbuf, or merged tpsb transposes
    (v41 corrupt with all three reverted).
  - NOT the ACT-copy/Pool-add merge (v36 corrupt with DVE stt merge).
  - NOT a cross-batch-overlap-only effect: v47 (inter-batch empty-For_i
    barrier) still corrupt 4/4 trials.
  - NOT the N=1 ones-matmul rowsum alone: v48 (64-wide ones) still
    corrupt 1/5 trials (others at baseline 1.3e-3).
  - Wider qn/kn pool (8 bufs) did not help either.
  RESOLVED: v26 (qkt_dma, no spread) bit-stable across trials while
  v27 (+spread_t) corrupts to 5e-2 => the spread_t queue layout is the
  race; qkt_dma itself is innocent. Synthesis over all runs: corruption
  amplitude tracks how many XBAR transposes are issued from the SP
  queue (v32 family with Qt+Kt on SP worst; v42 with only Qt mild).
"""

import sys

for _p in ("/opt/trn_rl_repo", "/root/.axon_site/_ro/trn_rl_repo"):
    if _p not in sys.path:
        sys.path.append(_p)

import numpy as np

B, N, E = 32, 2048, 512
NCORES = 8
NB = B // NCORES  # batches per core
NE = N // 128  # 16 token chunks
ED = E // 128  # 4 feature chunks
SCALE = 1.0 / float(np.sqrt(E))

_NC_CACHE = {}


def build_nc(nb=NB, variant=13, reps=1):
    """Build (once) the single-core Bass module processing nb batches.

    variant 1: all six projection layouts via fp32r matmuls from xT.
    variant 2: like 1, but Vt comes from a bf16 DMA(XBAR)-transpose of V
               instead of its own matmul projection (-64 matmuls/batch).
    variant 3: like 2, and Qt/Kt also come from DMA-transposes of the bf16
               natural q/k (row-attention S then runs in bf16;
               -128 more matmuls/batch).
    variant 4: like 3, plus V/Acol/AcolT double-buffered across batches so
               batch b+1's projection phase (PE) can stream while batch b's
               row attention still reads V.
    variant 5: variant 2 + early transposes: exp quarters are transposed
               unnormalized as soon as they exist, and the 1/rowsum lands on
               the PV output (per-partition ACT scale) instead of on P~;
               PV and the col output use separate PSUM tiles.
    variant 6: variant 5 + the double-buffering of variant 4.
    variant 7: variant 6 + 3-deep prow pool.
    """
    FEAT = {
        1: set(),
        2: {"vt_dma"},
        3: {"vt_dma", "qkt_dma"},
        4: {"vt_dma", "qkt_dma", "dbuf"},
        5: {"vt_dma", "early_t"},
        6: {"vt_dma", "early_t", "dbuf"},
        7: {"vt_dma", "early_t", "dbuf", "prow3"},
        8: {"vt_dma", "spread"},
        9: {"vt_dma", "early_t", "spread"},
        10: {"vt_dma", "late_norm", "act_drain"},
        11: {"vt_dma", "act_drain"},
        13: {"vt_dma", "late_norm", "pe_pt", "dve_merge"},
        14: {"vt_dma", "late_norm", "dve_merge"},
        # timing-only diagnostics (wrong results): fake / absent pt transposes
        15: {"vt_dma", "fake_t"},
        16: {"vt_dma", "no_t"},
        17: {"late_norm", "pe_pt", "dve_merge"},
        18: {"vt_dma", "late_norm", "pe_pt", "dve_merge", "prow3"},
        20: {"vt_dma", "late_norm", "pe_pt", "dve_merge", "psum2"},
        22: {"late_norm", "pe_pt", "dve_merge", "f32r_pv", "slice_xt"},
        23: {"late_norm", "pe_pt", "dve_merge", "f32r_pv", "slice_xt", "prow3"},
        26: {"vt_dma", "qkt_dma", "late_norm", "pe_pt", "dve_merge"},
        27: {"vt_dma", "qkt_dma", "late_norm", "pe_pt", "dve_merge", "spread_t"},
        28: {"vt_dma", "qkt_dma", "late_norm", "pe_pt", "dve_merge", "spread_t", "dbuf"},
        # 30: St-direct row attention: S^T = K Q^T computed in [m-part, j]
        # layout per 512-token j-group, so exp() lands directly in P^T layout
        # (no P transposes at all); row sums via a ones-column matmul that
        # reuses the PV stationary; PV normalization applied at the merge.
        30: {"vt_dma", "qkt_dma", "spread_t", "st_direct"},
        # 47: v30 + a hard all-engine barrier between batches. Every corrupt
        # run of the 30+ line had batch 0 clean (the race needs cross-batch
        # overlap); the barrier removes that surface for ~60-90us of model
        # time, keeping the ~2x real marginal-time win over variant 13.
        47: {"vt_dma", "qkt_dma", "spread_t", "st_direct", "bbar"},
        # 48: v30 with the rowsum ones-matmul widened from N=1 to N=64
        # moving columns (tiny-N matmul suspected as the HW-only corruption;
        # it is the one mechanism present in every corrupt variant and
        # absent from stable variant 13).
        48: {"vt_dma", "qkt_dma", "spread_t", "st_direct", "ones64"},
        # 50: st_direct on v13's exact queue layout (no spread_t): x and y on
        # sync, all XBAR transposes on scalar — tests whether the corruption
        # is tied to the spread_t queue assignment (e.g. transposes on SP
        # colliding with the tile framework's sem housekeeping).
        50: {"vt_dma", "qkt_dma", "st_direct"},
        # 51: st_direct with ALL XBAR transposes on the scalar queue and x/y
        # on gpsimd — the synthesis of every clean/corrupt observation is
        # that transposes issued from the SP queue are the toxin.
        51: {"vt_dma", "qkt_dma", "st_direct", "spread_t2"},
        # 52: + Qt/Kt transposes issued before Vt per chunk, and all Vt
        # transposes deferred to phase-A end: B1 needs Kt chunks ASAP while
        # Vt is only consumed by B2 — pure emission-order change on the same
        # (race-safe) scalar queue.
        52: {"vt_dma", "qkt_dma", "st_direct", "spread_t2", "vt_late"},
        # race-safe buffering variants of 51 (pool-depth only):
        53: {"vt_dma", "qkt_dma", "st_direct", "spread_t2", "prow3"},
        # 56: Vt via PE transposes (v13's proven pe_pt machinery) instead of
        # the DMA XBAR, halving the scalar transpose queue that feeds B1.
        56: {"qkt_dma", "st_direct", "spread_t2", "pe_vt"},
        54: {"vt_dma", "qkt_dma", "st_direct", "spread_t2", "slice_xt", "dbuf"},
        55: {"vt_dma", "qkt_dma", "st_direct", "spread_t2", "slice_xt", "dbuf", "prow3"},
        # 31: + phase-A rebalance: per-chunk xT slices with a single merged
        # PSUM tile + one DVE copy, V-drain on Pool, Vn/AcolT double-buffered.
        31: {"vt_dma", "qkt_dma", "spread_t", "st_direct", "xsl2", "vdrain_pool", "dbuf2"},
        # 32: restructured batch body: software-pipelined phase A (projections
        # one chunk behind transposes), qn/kn stored full so col-S runs as an
        # A-end block on only 2 PSUM slots, Qt/Kt XBAR transposes on the sync
        # queue (no WAR head-of-line block at batch boundaries), Vt/AcolT on
        # scalar at A-end, x/y DMA on gpsimd.
        32: {"v32"},
        33: {"v32", "xf32r"},
        34: {"v32", "wdirect"},
        35: {"v32", "wdirect", "xf32r"},
        # 36: v32 structure but the proven DVE stt merge (no ACT scaled copy,
        # no Pool add) — isolates the HW divergence seen in 32/35.
        36: {"v32", "dve_stt"},
        37: {"v32", "dve_stt", "wdirect", "xf32r"},
        # bisection of the v32-on-HW corruption:
        38: {"v32", "dve_stt", "prow2"},           # restore ptg double-buffer
        39: {"v32", "dve_stt", "nodbuf2"},         # drop Vn/Vt double-buffer
        40: {"v32", "dve_stt", "tps_sep"},         # v30-style separate transposes
        41: {"v32", "dve_stt", "prow2", "nodbuf2", "tps_sep"},
        # bisect from the v30 side: 42 = v30-equivalent inside the v32 body
        42: {"v32", "dve_stt", "prow2", "nodbuf2", "tps_sep", "cols_inline",
             "octmp_dve", "seq_a", "kt_scalar"},
        43: {"v32", "dve_stt", "prow2", "nodbuf2", "tps_sep", "cols_inline",
             "octmp_dve", "kt_scalar"},                       # +pipelined A
        44: {"v32", "dve_stt", "prow2", "nodbuf2", "tps_sep", "cols_inline",
             "octmp_dve", "seq_a"},                           # +Kt on sync
        45: {"v32", "dve_stt", "prow2", "nodbuf2", "tps_sep",
             "octmp_dve", "seq_a", "kt_scalar"},              # +colS-late+defer
        46: {"v32", "dve_stt", "prow2", "nodbuf2", "tps_sep", "cols_inline",
             "seq_a", "kt_scalar"},                           # +octmp on ACT
    }
    feat = FEAT[variant]
    if (nb, variant, reps) in _NC_CACHE:
        return _NC_CACHE[(nb, variant, reps)]

    import concourse.bass as bass
    import concourse.tile as tile
    from concourse import bacc, mybir
    from concourse.masks import make_identity

    f32 = mybir.dt.float32
    f32r = mybir.dt.float32r
    bf16 = mybir.dt.bfloat16
    AF = mybir.ActivationFunctionType
    AX = mybir.AxisListType

    nc = bacc.Bacc("TRN2")
    xdt = f32r if "xf32r" in feat else f32
    wdt = f32r if "wdirect" in feat else f32
    x = nc.dram_tensor("x", [nb, N, E], xdt, kind="ExternalInput")
    w = nc.dram_tensor("w", [E, 3 * E], wdt, kind="ExternalInput")
    bvec = nc.dram_tensor("b", [3 * E], f32, kind="ExternalInput")
    y = nc.dram_tensor("y", [nb, N, E], f32, kind="ExternalOutput")

    with tile.TileContext(nc) as tc:
        with (
            tc.tile_pool(name="const", bufs=1) as constp,
            tc.tile_pool(name="xn", bufs=2 if ("prow3" in feat or "slice_xt" in feat) else 3) as xnp,
            tc.tile_pool(name="big", bufs=1) as bigp,
            tc.tile_pool(name="xsl", bufs=2) as xslp,
            tc.tile_pool(name="qkn", bufs=8 if "st_direct" in feat else 3) as qknp,
            tc.tile_pool(name="prow", bufs=3 if "prow3" in feat else (2 if ("prow2" in feat or "v32" not in feat) else 1)) as prowp,
            tc.tile_pool(name="stat", bufs=3) as statp,
            tc.tile_pool(name="outp", bufs=2) as outpp,
            tc.tile_pool(name="ps_proj", bufs=2, space="PSUM") as ps_proj,
            tc.tile_pool(name="ps_sc", bufs=4, space="PSUM") as ps_sc,
            tc.tile_pool(name="ps_s", bufs=2, space="PSUM") as ps_s,
        ):
            # ---------------- constants ----------------
            # W lands as float32r (rounded by the DVE copy) so fp32r matmuls
            # accept it; staged through the small xn pool to save SBUF.
            W = constp.tile([128, ED, 3 * E], f32r)
            wv = w[:].rearrange("(k p) m -> p k m", p=128)
            if "wdirect" in feat:
                # f32r is fp32 bits: DMA W straight in, no staging copies.
                for k in range(ED):
                    nc.sync.dma_start(W[:, k, :], wv[:, k, :])
            else:
                for k in range(ED):
                    for c in range(3):
                        wst = xnp.tile([128, E], f32, tag="xn", name=f"wst{k}_{c}")
                        nc.sync.dma_start(wst, wv[:, k, c * E : (c + 1) * E])
                        nc.vector.tensor_copy(W[:, k, c * E : (c + 1) * E], wst)

            # bias broadcast across partitions (for [n-part, e] layouts)
            b3 = bvec[:].rearrange("(c m) -> c m", m=E)
            bb = constp.tile([128, 3, E], bf16)
            nc.gpsimd.dma_start(
                bb, bass.AP(tensor=b3.tensor, offset=b3.offset, ap=[[0, 128]] + list(b3.ap))
            )
            # bias per partition (for [e-part, n] layouts): bpart[p, c] = b[c*128+p]
            bpart = constp.tile([128, 3 * ED], f32)
            nc.gpsimd.dma_start(bpart, bvec[:].rearrange("(c p) -> p c", p=128))

            ident = constp.tile([128, 128], f32)
            make_identity(nc, ident)
            identB = constp.tile([128, 128], bf16)
            make_identity(nc, identB)
            ones = constp.tile([128, 1], bf16)
            nc.gpsimd.memset(ones, 1.0)
            ones64 = constp.tile([128, 64], bf16)
            nc.gpsimd.memset(ones64, 1.0)
            if "xf32r" in feat:
                # gpsimd memset can't target f32r (invalid ISA) -> copy from
                # the f32 identity instead.
                identX = constp.tile([128, 128], f32r)
                nc.vector.tensor_copy(identX, ident)
            else:
                identX = ident

            def batch_body_v32():
              for b in range(nb):
                Qt = bigp.tile([128, ED, N], bf16, tag="Qt")
                Kt = bigp.tile([128, ED, N], bf16, tag="Kt")
                qf = bigp.tile([128, NE, E], bf16, tag="qf")
                kf = bigp.tile([128, NE, E], bf16, tag="kf")
                # Vn/Vt double-buffered: their writers would otherwise WAR
                # against the previous batch's B2 reads and head-of-line-block
                # the DVE / transpose queues right at the batch boundary.
                vbufs = 1 if "nodbuf2" in feat else 2
                Vn = bigp.tile([128, NE, E], bf16, tag="Vn", bufs=vbufs)
                Vt = bigp.tile([128, ED, N], bf16, tag="Vt", bufs=vbufs)
                Acol = bigp.tile([128, ED, E], bf16, tag="Acol")
                AcolT = bigp.tile([128, ED, E], bf16, tag="AcolT")

                # ---- phase A: chunk pipeline (projections one chunk behind
                # the PE transposes so the PSUM->SBUF copy is off the PE path)
                def emit_load(j):
                    xn = xnp.tile([128, E], xdt, tag="xn")
                    nc.gpsimd.dma_start(xn, x[b, j * 128 : (j + 1) * 128, :])
                    xsl = xslp.tile([128, ED, 128], f32r, tag="xsl", name=f"xsl{b}_{j}")
                    if "tps_sep" in feat:
                        for k in range(ED):
                            tps = ps_proj.tile([128, 128], xdt, tag="ps", name=f"tps{b}_{j}_{k}")
                            nc.tensor.transpose(
                                tps, xn[:, k * 128 : (k + 1) * 128], identX
                            )
                            nc.vector.tensor_copy(xsl[:, k, :], tps)
                    else:
                        tpsb = ps_proj.tile([128, E], xdt, tag="ps")
                        for k in range(ED):
                            nc.tensor.transpose(
                                tpsb[:, k * 128 : (k + 1) * 128],
                                xn[:, k * 128 : (k + 1) * 128],
                                identX,
                            )
                        nc.vector.tensor_copy(xsl.rearrange("p a b -> p (a b)"), tpsb)
                    return xsl

                def emit_proj(j, xsl):
                    jsl = slice(j * 128, (j + 1) * 128)
                    for dst, ci, deng in (
                        (qf[:, j, :], 0, nc.vector),
                        (kf[:, j, :], 1, nc.vector),
                        (Vn[:, j, :], 2, nc.vector),
                    ):
                        pp = ps_proj.tile([128, E], f32, tag="ps")
                        for k in range(ED):
                            nc.tensor.matmul(
                                pp,
                                xsl[:, k, :],
                                W[:, k, ci * E : (ci + 1) * E],
                                start=(k == 0),
                                stop=(k == ED - 1),
                            )
                        deng.tensor_add(dst, pp, bb[:, ci, :])
                    nc.sync.dma_start_transpose(Qt[:, :, jsl], qf[:, j, :])
                    kteng = nc.scalar if "kt_scalar" in feat else nc.sync
                    kteng.dma_start_transpose(Kt[:, :, jsl], kf[:, j, :])
                    vteng = (
                        (nc.sync if j % 2 else nc.scalar)
                        if "kt_scalar" in feat
                        else nc.scalar
                    )
                    vteng.dma_start_transpose(Vt[:, :, jsl], Vn[:, j, :])

                if "seq_a" in feat:
                    for j in range(NE):
                        emit_proj(j, emit_load(j))
                else:
                    prev = None
                    for j in range(NE):
                        xsl_j = emit_load(j)
                        if prev is not None:
                            emit_proj(*prev)
                        prev = (j, xsl_j)
                    emit_proj(*prev)

                # ---- col-S as an A-end block: 2 PSUM slots at a time, so the
                # previous batch's B2 po/oc can keep cycling the other 2.
                cstat = statp.tile([128, 3 * ED], f32, tag="cstat")

                def emit_colS_half(half):
                    cs = [
                        ps_sc.tile([128, E], f32, tag="scps", name=f"cs{b}_{half}_{i2}")
                        for i2 in range(2)
                    ]
                    for c in range(NE):
                        for i2 in range(2):
                            i = half * 2 + i2
                            nc.tensor.matmul(
                                cs[i2],
                                qf[:, c, i * 128 : (i + 1) * 128],
                                kf[:, c, :],
                                start=(c == 0),
                                stop=(c == NE - 1),
                            )
                    return cs

                def emit_colsm_half(half, cs):
                    for i2 in range(2):
                        i = half * 2 + i2
                        nm = cstat[:, 2 * ED + i : 2 * ED + i + 1]
                        nc.vector.reduce_max(nm, cs[i2], axis=AX.X, negate=True)
                        nc.vector.tensor_scalar_mul(nm, nm, SCALE)
                        nc.scalar.activation(
                            out=Acol[:, i, :],
                            in_=cs[i2],
                            func=AF.Exp,
                            scale=SCALE,
                            bias=nm,
                            accum_out=cstat[:, i : i + 1],
                        )

                def emit_colsm_finish():
                    nc.vector.reciprocal(cstat[:, ED : 2 * ED], cstat[:, 0:ED])
                    for i in range(ED):
                        nc.vector.tensor_scalar_mul(
                            Acol[:, i, :], Acol[:, i, :], cstat[:, ED + i : ED + i + 1]
                        )
                        nc.scalar.dma_start_transpose(
                            AcolT[:, :, i * 128 : (i + 1) * 128], Acol[:, i, :]
                        )

                if "cols_inline" in feat:
                    csall = [
                        ps_sc.tile([128, E], f32, tag="scps", name=f"cs{b}_{i}")
                        for i in range(ED)
                    ]
                    for c in range(NE):
                        for i in range(ED):
                            nc.tensor.matmul(
                                csall[i],
                                qf[:, c, i * 128 : (i + 1) * 128],
                                kf[:, c, :],
                                start=(c == 0),
                                stop=(c == NE - 1),
                            )
                    emit_colsm_half(0, csall[0:2])
                    emit_colsm_half(1, csall[2:4])
                    emit_colsm_finish()
                    cs1 = None
                else:
                    cs0 = emit_colS_half(0)
                    emit_colsm_half(0, cs0)
                    cs1 = emit_colS_half(1)
                # A2 for the second half is deferred into B1 g0 (below) so the
                # first St exps aren't queued behind the col-softmax on ACT.

                # ---- phase B: identical structure to st_direct
                for g in range(4):
                    gsl = slice(g * 512, (g + 1) * 512)
                    ptg = prowp.tile([128, NE, 512], bf16, tag="ptg")
                    for m in range(NE):
                        if g == 0 and m == 4 and cs1 is not None:
                            # deferred col-softmax half 1: its ACT exps queue
                            # behind B1's first St exps instead of ahead.
                            emit_colsm_half(1, cs1)
                            emit_colsm_finish()
                        sps = ps_s.tile([128, 512], f32, tag="s")
                        for k in range(ED):
                            nc.tensor.matmul(
                                sps,
                                Kt[:, k, m * 128 : (m + 1) * 128],
                                Qt[:, k, gsl],
                                start=(k == 0),
                                stop=(k == ED - 1),
                            )
                        nc.scalar.activation(
                            out=ptg[:, m, :], in_=sps, func=AF.Exp, scale=SCALE
                        )
                    ocs = {}
                    def emit_oc(j):
                        oc = ps_sc.tile([128, E], f32, tag="scps", name=f"oc{b}_{j}")
                        for c in range(ED):
                            nc.tensor.matmul(
                                oc,
                                Vt[:, c, j * 128 : (j + 1) * 128],
                                AcolT[:, c, :],
                                start=(c == 0),
                                stop=(c == ED - 1),
                            )
                        ocs[j] = oc
                    emit_oc(g * 4)
                    emit_oc(g * 4 + 1)
                    for jj in range(4):
                        j = g * 4 + jj
                        jpart = slice(jj * 128, (jj + 1) * 128)
                        po = ps_sc.tile([128, E], f32, tag="scps")
                        rs = ps_s.tile([128, 8], f32, tag="s", name=f"rs{b}_{j}")
                        for m in range(NE):
                            nc.tensor.matmul(
                                po,
                                ptg[:, m, jpart],
                                Vn[:, m, :],
                                start=(m == 0),
                                stop=(m == NE - 1),
                            )
                            nc.tensor.matmul(
                                rs[:, 0:1],
                                ptg[:, m, jpart],
                                ones,
                                start=(m == 0),
                                stop=(m == NE - 1),
                            )
                        rstat = statp.tile([128, 8], f32, tag="rstat")
                        nc.vector.reciprocal(rstat[:, 5:6], rs[:, 0:1])
                        if jj < 2:
                            emit_oc(g * 4 + 2 + jj)
                        ot = outpp.tile([128, E], f32, tag="ot")
                        octmp = outpp.tile([128, E], f32, tag="octmp")
                        if "dve_stt" in feat:
                            if "octmp_dve" in feat:
                                nc.vector.tensor_copy(octmp, ocs.pop(j))
                            else:
                                nc.scalar.activation(
                                    out=octmp, in_=ocs.pop(j), func=AF.Copy
                                )
                            nc.vector.scalar_tensor_tensor(
                                ot,
                                po,
                                rstat[:, 5:6],
                                octmp,
                                op0=mybir.AluOpType.mult,
                                op1=mybir.AluOpType.add,
                            )
                        else:
                            potmp = outpp.tile([128, E], f32, tag="potmp")
                            # merge off DVE entirely: ACT drains oc and the
                            # 1/rowsum-scaled po (per-partition scale), Pool
                            # does the SBUF-only add.
                            nc.scalar.activation(
                                out=octmp, in_=ocs.pop(j), func=AF.Copy
                            )
                            nc.scalar.activation(
                                out=potmp, in_=po, func=AF.Copy, scale=rstat[:, 5:6]
                            )
                            nc.gpsimd.tensor_add(ot, potmp, octmp)
                        nc.gpsimd.dma_start(y[b, j * 128 : (j + 1) * 128, :], ot)

            def batch_body():
              if "v32" in feat:
                  return batch_body_v32()
              for b in range(nb):
                qkt_dt = bf16 if "qkt_dma" in feat else f32r
                vn_dt = f32r if "f32r_pv" in feat else bf16
                dbufs = 2 if "dbuf" in feat else 1
                dbufs2 = 2 if "dbuf2" in feat else dbufs
                slice_xt = "slice_xt" in feat or "xsl2" in feat
                if not slice_xt:
                    xT = bigp.tile([128, ED, N], f32r, tag="xT")
                Qt = bigp.tile([128, ED, N], qkt_dt, tag="Qt")
                Kt = bigp.tile([128, ED, N], qkt_dt, tag="Kt")
                Vn = bigp.tile([128, NE, E], vn_dt, tag="Vn", bufs=dbufs2)
                Vt = bigp.tile([128, ED, N], bf16, tag="Vt")
                Acol = bigp.tile([128, ED, E], bf16, tag="Acol", bufs=dbufs)
                AcolT = bigp.tile([128, ED, E], bf16, tag="AcolT", bufs=dbufs2)
                scol_ps = [
                    ps_sc.tile([128, E], f32, tag="scps", name=f"scol{b}_{i}")
                    for i in range(ED)
                ]

                # ---- phase A: load x, build xT, projections, col-S accumulation
                for j in range(NE):
                    s_idx, jj = j // ED, j % ED
                    if "xsl2" in feat:
                        xT = xslp.tile(
                            [128, ED, 128], f32r, tag="xsl", name=f"xsl{b}_{j}"
                        )
                        xoff, roff = 0, 0
                    elif slice_xt:
                        if jj == 0:
                            xT = xslp.tile(
                                [128, ED, ED * 128], f32r, tag="xsl", name=f"xsl{b}_{s_idx}"
                            )
                        xoff, roff = jj * 128, 0
                    else:
                        xoff, roff = j * 128, s_idx * 512
                    xn = xnp.tile([128, E], f32, tag="xn")
                    xeng = (
                        nc.gpsimd
                        if ("spread_t" in feat or "spread_t2" in feat)
                        else nc.sync
                    )
                    xeng.dma_start(xn, x[b, j * 128 : (j + 1) * 128, :])
                    if "xsl2" in feat:
                        # all 4 transposes land in one PSUM tile -> one copy
                        tpsb = ps_proj.tile([128, E], f32, tag="ps")
                        for k in range(ED):
                            nc.tensor.transpose(
                                tpsb[:, k * 128 : (k + 1) * 128],
                                xn[:, k * 128 : (k + 1) * 128],
                                ident,
                            )
                        nc.vector.tensor_copy(
                            xT.rearrange("p a b -> p (a b)"), tpsb
                        )
                    else:
                        for k in range(ED):
                            tps = ps_proj.tile([128, 128], f32, tag="ps")
                            nc.tensor.transpose(tps, xn[:, k * 128 : (k + 1) * 128], ident)
                            nc.vector.tensor_copy(xT[:, k, xoff : xoff + 128], tps)

                    # natural-layout q, k, v for this token chunk
                    qn = qknp.tile([128, E], bf16, tag="qn")
                    kn = qknp.tile([128, E], bf16, tag="kn")
                    for dst, ci in ((qn, 0), (kn, 1), (Vn[:, j, :], 2)):
                        pp = ps_proj.tile([128, E], f32, tag="ps")
                        for k in range(ED):
                            nc.tensor.matmul(
                                pp,
                                xT[:, k, xoff : xoff + 128],
                                W[:, k, ci * E : (ci + 1) * E],
                                start=(k == 0),
                                stop=(k == ED - 1),
                            )
                        deng = (
                            nc.gpsimd
                            if (ci == 2 and "vdrain_pool" in feat)
                            else nc.vector
                        )
                        deng.tensor_add(dst, pp, bb[:, ci, :])

                    # col-attention S accumulation: S_col[d,e] += q_j.T @ k_j
                    for i in range(ED):
                        nc.tensor.matmul(
                            scol_ps[i],
                            qn[:, i * 128 : (i + 1) * 128],
                            kn,
                            start=(j == 0),
                            stop=(j == NE - 1),
                        )

                    # bf16 transposed layouts via the DMA XBAR (free wrt PE)
                    jsl = slice(j * 128, (j + 1) * 128)
                    if "pe_vt" in feat:
                        for kq in range(ED):
                            vps = ps_proj.tile(
                                [128, 128], bf16, tag="ps", name=f"vps{b}_{j}_{kq}"
                            )
                            nc.tensor.transpose(
                                vps, Vn[:, j, kq * 128 : (kq + 1) * 128], identB
                            )
                            nc.vector.tensor_copy(Vt[:, kq, jsl], vps)
                    if "qkt_dma" in feat and "vt_late" in feat:
                        # B1 consumes Kt/Qt first: put them at the queue head
                        nc.scalar.dma_start_transpose(Qt[:, :, jsl], qn)
                        nc.scalar.dma_start_transpose(Kt[:, :, jsl], kn)
                    if "vt_dma" in feat and "vt_late" not in feat:
                        veng = (
                            (nc.sync if j % 2 else nc.scalar)
                            if "spread_t" in feat
                            else nc.scalar
                        )
                        veng.dma_start_transpose(Vt[:, :, jsl], Vn[:, j, :])
                    if "qkt_dma" in feat and "vt_late" not in feat:
                        qeng = nc.sync if "spread_t" in feat else nc.scalar
                        qeng.dma_start_transpose(Qt[:, :, jsl], qn)
                        nc.scalar.dma_start_transpose(Kt[:, :, jsl], kn)

                    # transposed-layout projections, one 512-token slice at a time
                    if "qkt_dma" in feat:
                        tproj = ()
                    elif "vt_dma" in feat:
                        tproj = ((Qt, 0), (Kt, 1))
                    else:
                        tproj = ((Qt, 0), (Kt, 1), (Vt, 2))
                    if j % ED == ED - 1 and tproj:
                        sl = slice(s_idx * 512, (s_idx + 1) * 512)
                        for dst, ci in tproj:
                            for i in range(ED):
                                pp = ps_proj.tile([128, E], f32, tag="ps")
                                for k in range(ED):
                                    nc.tensor.matmul(
                                        pp,
                                        W[:, k, ci * E + i * 128 : ci * E + (i + 1) * 128],
                                        xT[:, k, roff : roff + 512],
                                        start=(k == 0),
                                        stop=(k == ED - 1),
                                    )
                                if "act_drain" in feat:
                                    nc.scalar.activation(
                                        out=dst[:, i, sl],
                                        in_=pp,
                                        func=AF.Identity,
                                        bias=bpart[:, ci * ED + i : ci * ED + i + 1],
                                    )
                                else:
                                    nc.vector.tensor_scalar_add(
                                        dst[:, i, sl], pp, bpart[:, ci * ED + i : ci * ED + i + 1]
                                    )

                # ---- phase A2: col softmax + transpose of A
                # col logits are O(+-600): subtract the per-row max (as an ACT
                # bias of -max*SCALE) before exp, unlike the row path.
                cstat = statp.tile([128, 3 * ED], f32, tag="cstat")
                for i in range(ED):
                    nm = cstat[:, 2 * ED + i : 2 * ED + i + 1]
                    nc.vector.reduce_max(nm, scol_ps[i], axis=AX.X, negate=True)
                    nc.vector.tensor_scalar_mul(nm, nm, SCALE)
                    nc.scalar.activation(
                        out=Acol[:, i, :],
                        in_=scol_ps[i],
                        func=AF.Exp,
                        scale=SCALE,
                        bias=nm,
                        accum_out=cstat[:, i : i + 1],
                    )
                nc.vector.reciprocal(cstat[:, ED : 2 * ED], cstat[:, 0:ED])
                for i in range(ED):
                    nc.vector.tensor_scalar_mul(
                        Acol[:, i, :], Acol[:, i, :], cstat[:, ED + i : ED + i + 1]
                    )
                    nc.scalar.dma_start_transpose(
                        AcolT[:, :, i * 128 : (i + 1) * 128], Acol[:, i, :]
                    )

                if "vt_late" in feat:
                    for jv in range(NE):
                        nc.scalar.dma_start_transpose(
                            Vt[:, :, jv * 128 : (jv + 1) * 128], Vn[:, jv, :]
                        )

                # ---- phase B (st_direct): S^T = K Q^T per 512-token j-group;
                # exp() output IS P~^T (no transposes); row sums via a ones
                # column reusing the PV stationary; 1/rowsum at the merge.
                if "st_direct" in feat:
                    for g in range(4):
                        gsl = slice(g * 512, (g + 1) * 512)
                        ptg = prowp.tile([128, NE, 512], bf16, tag="ptg")
                        for m in range(NE):
                            sps = ps_s.tile([128, 512], f32, tag="s")
                            for k in range(ED):
                                nc.tensor.matmul(
                                    sps,
                                    Kt[:, k, m * 128 : (m + 1) * 128],
                                    Qt[:, k, gsl],
                                    start=(k == 0),
                                    stop=(k == ED - 1),
                                )
                            nc.scalar.activation(
                                out=ptg[:, m, :], in_=sps, func=AF.Exp, scale=SCALE
                            )
                        # col-attention output for this group's 4 chunks: fills
                        # the PE while the last exp() quarters drain.
                        ocs = {}
                        def emit_oc(j):
                            oc = ps_sc.tile(
                                [128, E], f32, tag="scps", name=f"oc{b}_{j}"
                            )
                            for c in range(ED):
                                nc.tensor.matmul(
                                    oc,
                                    Vt[:, c, j * 128 : (j + 1) * 128],
                                    AcolT[:, c, :],
                                    start=(c == 0),
                                    stop=(c == ED - 1),
                                )
                            ocs[j] = oc
                        emit_oc(g * 4)
                        emit_oc(g * 4 + 1)
                        for jj in range(4):
                            j = g * 4 + jj
                            jpart = slice(jj * 128, (jj + 1) * 128)
                            po = ps_sc.tile([128, E], f32, tag="scps")
                            rs = ps_s.tile(
                                [128, 64 if "ones64" in feat else 8], f32,
                                tag="s", name=f"rs{b}_{j}",
                            )
                            onesv = ones64 if "ones64" in feat else ones
                            rsl = slice(0, 64) if "ones64" in feat else slice(0, 1)
                            for m in range(NE):
                                nc.tensor.matmul(
                                    po,
                                    ptg[:, m, jpart],
                                    Vn[:, m, :],
                                    start=(m == 0),
                                    stop=(m == NE - 1),
                                )
                                nc.tensor.matmul(
                                    rs[:, rsl],
                                    ptg[:, m, jpart],
                                    onesv,
                                    start=(m == 0),
                                    stop=(m == NE - 1),
                                )
                            rstat = statp.tile([128, 8], f32, tag="rstat")
                            nc.vector.reciprocal(rstat[:, 5:6], rs[:, 0:1])
                            if jj < 2:
                                emit_oc(g * 4 + 2 + jj)
                            ot = outpp.tile([128, E], f32, tag="ot")
                            octmp = outpp.tile([128, E], f32, tag="octmp")
                            nc.vector.tensor_copy(octmp, ocs.pop(j))
                            nc.vector.scalar_tensor_tensor(
                                ot,
                                po,
                                rstat[:, 5:6],
                                octmp,
                                op0=mybir.AluOpType.mult,
                                op1=mybir.AluOpType.add,
                            )
                            yeng2 = (
                                nc.gpsimd
                                if ("spread_t" in feat or "spread_t2" in feat)
                                else nc.sync
                            )
                            yeng2.dma_start(
                                y[b, j * 128 : (j + 1) * 128, :], ot
                            )
                    if "bbar" in feat and b < nb - 1:
                        # empty 1-iteration hardware loop = supported
                        # all-engine barrier between batches
                        with tc.For_i(0, 1, 1):
                            pass
                    continue

                # ---- phase B: row attention + merged output, per token chunk
                early_t = "early_t" in feat
                late_norm = "late_norm" in feat
                spread = "spread" in feat
                for j in range(NE):
                    teng = (nc.sync if j % 2 else nc.scalar) if spread else nc.scalar
                    yeng = (nc.scalar if j % 2 else nc.sync) if spread else nc.sync
                    if "spread_t" in feat:
                        yeng = nc.gpsimd
                    pt = prowp.tile([128, N], bf16, tag="pt")
                    ptT = prowp.tile(
                        [128, NE, 128], f32r if "f32r_pv" in feat else bf16, tag="ptT"
                    )
                    rstat = statp.tile([128, 8], f32, tag="rstat")
                    for q in range(4):
                        sps = ps_s.tile([128, 512], f32, tag="s")
                        for k in range(ED):
                            nc.tensor.matmul(
                                sps,
                                Qt[:, k, j * 128 : (j + 1) * 128],
                                Kt[:, k, q * 512 : (q + 1) * 512],
                                start=(k == 0),
                                stop=(k == ED - 1),
                            )
                        nc.scalar.activation(
                            out=pt[:, q * 512 : (q + 1) * 512],
                            in_=sps,
                            func=AF.Exp,
                            scale=SCALE,
                            accum_out=rstat[:, q : q + 1],
                        )
                        if early_t:
                            # transpose the unnormalized quarter right away;
                            # 1/rowsum is applied to the PV output instead
                            teng.dma_start_transpose(
                                ptT[:, 4 * q : 4 * q + 4, :],
                                pt[:, q * 512 : (q + 1) * 512],
                            )
                        if "pe_pt" in feat:
                            for t in range(4):
                                m = 4 * q + t
                                psB = ps_proj.tile(
                                    [128, 128], bf16, tag="ps", name=f"psB{b}_{j}_{m}"
                                )
                                nc.tensor.transpose(
                                    psB, pt[:, m * 128 : (m + 1) * 128], identB
                                )
                                nc.vector.tensor_copy(ptT[:, m, :], psB)
                    nc.vector.reduce_sum(rstat[:, 4:5], rstat[:, 0:4], axis=AX.X)
                    nc.vector.reciprocal(rstat[:, 5:6], rstat[:, 4:5])
                    if late_norm:
                        if "pe_pt" not in feat:
                            teng.dma_start_transpose(ptT, pt)
                    elif not early_t:
                        nc.vector.tensor_scalar_mul(pt, pt, rstat[:, 5:6])
                        if "fake_t" in feat:
                            teng.dma_start(ptT.rearrange("p a b -> p (a b)"), pt)
                        elif "no_t" in feat:
                            nc.vector.tensor_copy(ptT[:, 0, :], pt[:, :128])
                        else:
                            teng.dma_start_transpose(ptT, pt)

                    po = ps_sc.tile([128, E], f32, tag="scps")
                    for m in range(NE):
                        nc.tensor.matmul(
                            po,
                            ptT[:, m, :],
                            Vn[:, m, :],
                            start=(m == 0),
                            stop=((early_t or late_norm) and m == NE - 1),
                        )
                    ot = outpp.tile([128, E], f32, tag="ot")
                    if early_t or late_norm:
                        oc = ps_sc.tile([128, E], f32, tag="scps")
                        for c in range(ED):
                            nc.tensor.matmul(
                                oc,
                                Vt[:, c, j * 128 : (j + 1) * 128],
                                AcolT[:, c, :],
                                start=(c == 0),
                                stop=(c == ED - 1),
                            )
                        if "dve_merge" in feat:
                            if "psum2" in feat:
                                nc.vector.scalar_tensor_tensor(
                                    ot,
                                    po,
                                    rstat[:, 5:6],
                                    oc,
                                    op0=mybir.AluOpType.mult,
                                    op1=mybir.AluOpType.add,
                                )
                            else:
                                octmp = outpp.tile([128, E], f32, tag="octmp")
                                nc.vector.tensor_copy(octmp, oc)
                                nc.vector.scalar_tensor_tensor(
                                    ot,
                                    po,
                                    rstat[:, 5:6],
                                    octmp,
                                    op0=mybir.AluOpType.mult,
                                    op1=mybir.AluOpType.add,
                                )
                        else:
                            nc.scalar.activation(
                                out=ot, in_=po, func=AF.Copy, scale=rstat[:, 5:6]
                            )
                            nc.vector.tensor_add(ot, ot, oc)
                    else:
                        for c in range(ED):
                            nc.tensor.matmul(
                                po,
                                Vt[:, c, j * 128 : (j + 1) * 128],
                                AcolT[:, c, :],
                                start=False,
                                stop=(c == ED - 1),
                            )
                        nc.vector.tensor_copy(ot, po)
                    yeng.dma_start(y[b, j * 128 : (j + 1) * 128, :], ot)

            if reps == 1:
                batch_body()
            else:
                with tc.For_i(0, reps, 1):
                    batch_body()

    nc.compile()
    _NC_CACHE[(nb, variant, reps)] = nc
    return nc


def make_in_maps(x, w_qkv, b_qkv):
    xs = np.ascontiguousarray(np.asarray(x, dtype=np.float32)).reshape(B, N, E)
    w = np.ascontiguousarray(np.asarray(w_qkv, dtype=np.float32))
    bq = np.ascontiguousarray(np.asarray(b_qkv, dtype=np.float32))
    return [
        {"x": np.ascontiguousarray(xs[c * NB : (c + 1) * NB]), "w": w, "b": bq}
        for c in range(NCORES)
    ]


BEST_VARIANT = 51


def kernel(x, w_qkv, b_qkv):
    from concourse.bass_utils import run_bass_kernel_spmd

    nc = build_nc(NB, BEST_VARIANT)
    in_maps = make_in_maps(x, w_qkv, b_qkv)
    res = run_bass_kernel_spmd(nc, in_maps, core_ids=list(range(NCORES)))
    out = np.empty((B, N, E), dtype=np.float32)
    for c in range(NCORES):
        out[c * NB : (c + 1) * NB] = res.results[c]["y"]
    return out



# revision 2
# speedup vs baseline: 63.5933x; 63.5933x over previous
"""AxialSelfAttention Trainium2 Bass kernel.

Reference computation (per batch b):
    xs  = x[b] reshaped [N=2048, E=512]
    qkv = xs @ W + bias                      # [N, 3E]
    q, k, v = split(qkv)
    row:  P = softmax(q @ k.T / sqrt(E));  out_row = P @ v
    col:  A = softmax(q.T @ k / sqrt(E));  out_col = v @ A.T
    out = out_row + out_col                  # [N, E]

Sharding: data-parallel over batch B=32 across 8 cores (4 batches/core).

Shipped variant: 51 = the full St-direct algorithm on race-free queues:
row attention computed transposed (St = K Q^T per 512-token j-group) so
exp() lands directly in P^T layout (no P transposes, no DVE copies on
that path); Qt/Kt/Vt/AcolT via bf16 DMA-XBAR transposes ALL on the
scalar queue; x/y DMA on gpsimd; row sums via a ones-column matmul
reusing the PV stationary; 1/rowsum applied at the DVE merge.
Device-verified BIT-DETERMINISTIC at rel err 1.2320e-3 across 5
single-shot trials. Model 993us/core (v13: 1022, but with PE busy only
736us vs v13's 900us, half the DVE work, 17% fewer instructions);
MEASURED real marginal 1.14ms/rep (reps-delta, floor 66.4ms at
reps=33) vs v13's 1.93ms (-41%) and v26's 1.22ms.
(v26 = v13+qkt_dma, also bit-stable, measured 1.22ms/rep, is the
fallback if anything regresses.)

Real-device-time model, fitted on the two reps-delta measurements
(v26: 1.22ms @ 1084us-model/14033 insts; v30: ~0.95ms @ 850us/11913):
  real_per_rep ~= model_ns + 9.7ns * instruction_count
(validated: predicted 1.107ms for v51, measured 1.14ms — within 3%)
Under it the remaining screened candidates (52: flat; 54: -0.6% model;
56: -1.9% model but +973 insts) are all within noise of v51 (~1.10ms).
Real further gains need PE-work cuts (blocked: fp8 fails accuracy) or
the ~850us spread-queue schedule (blocked: SP-queue XBAR corruption).

The v50/v51 'deterministic 0.866' bug was NOT a framework issue: a
scripted edit had de-indented the y-store out of the jj-loop so only
j=3,7,11,15 were stored. Fixed; v51 then passed 5/5 bit-identical.

ROOT CAUSE of the long-chased corruption: the spread_t queue layout.
v26 (no spread) is bit-stable; v27 (= v26 + spread_t: Qt-T on the SP
queue, Vt-T alternating, x/y DMA on gpsimd) corrupts nondeterministically
up to 5e-2. Every corrupt variant (27-48) had spread_t; both clean
lines (13, 26) do not. Suspected mechanism: XBAR-transpose issues on the
SP queue colliding with the tile framework's semaphore housekeeping
(BassTileRelease runs on SP), or the gpsimd SWDGE x/y path.

st_direct follow-ups (v50/v51): moving the Qt/Vt transposes to the
scalar queue inside st_direct gives a DETERMINISTIC rel err 0.866 —
1536/2048 output rows all-zero (12 of 16 j-chunks never stored), same
value on device and in CoreSim (v51 restored x/y to gpsimd and it did
not change). Since it reproduces in CoreSim it is debuggable offline:
run CoreSim nb=1 on v50, find which j-chunks are unwritten and walk the
B2 store chain back (likely a framework scheduling/dep issue when all
52 XBAR transposes + 68 exps share the ACT queue). Fixing that unlocks
~968us-model st_direct on race-free queues -> est. ~1.0-1.1ms/rep.

Variant 30+ ("St-direct", see FEAT below) is the faster research line:
row attention computed transposed (St = K Q^T per 512-token j-group) so
exp() lands directly in P^T layout, Qt/Kt via DMA-XBAR transposes, row
sums via a ones-column matmul reusing the PV stationary. It models at
850us/core vs 13's 1022us and measured ~2x lower marginal device time
per rep (reps-delta: ~0.95ms vs 1.93ms), BUT it shows a nondeterministic
whole-chunk corruption on hardware (batch 0 always clean, later batches
intermittently 0.004-0.018 global rel err for the v30 family, 0.06-0.13
for the v32 family; CoreSim and TimelineSim both clean => HW-only).

Race-hunt results (all on device, single-shot kernel() trials):
  - NOT the ptg single-buffer, Vn/Vt dbuf, or merged tpsb transposes
    (v41 corrupt with all three reverted).
  - NOT the ACT-copy/Pool-add merge (v36 corrupt with DVE stt merge).
  - NOT a cross-batch-overlap-only effect: v47 (inter-batch empty-For_i
    barrier) still corrupt 4/4 trials.
  - NOT the N=1 ones-matmul rowsum alone: v48 (64-wide ones) still
    corrupt 1/5 trials (others at baseline 1.3e-3).
  - Wider qn/kn pool (8 bufs) did not help either.
  RESOLVED: v26 (qkt_dma, no spread) bit-stable across trials while
  v27 (+spread_t) corrupts to 5e-2 => the spread_t queue layout is the
  race; qkt_dma itself is innocent. Synthesis over all runs: corruption
  amplitude tracks how many XBAR transposes are issued from the SP
  queue (v32 family with Qt+Kt on SP worst; v42 with only Qt mild).
"""

import sys

for _p in ("/opt/trn_rl_repo", "/root/.axon_site/_ro/trn_rl_repo"):
    if _p not in sys.path:
        sys.path.append(_p)

import numpy as np

B, N, E = 32, 2048, 512
NCORES = 8
NB = B // NCORES  # batches per core
NE = N // 128  # 16 token chunks
ED = E // 128  # 4 feature chunks
SCALE = 1.0 / float(np.sqrt(E))

_NC_CACHE = {}


def build_nc(nb=NB, variant=13, reps=1):
    """Build (once) the single-core Bass module processing nb batches.

    variant 1: all six projection layouts via fp32r matmuls from xT.
    variant 2: like 1, but Vt comes from a bf16 DMA(XBAR)-transpose of V
               instead of its own matmul projection (-64 matmuls/batch).
    variant 3: like 2, and Qt/Kt also come from DMA-transposes of the bf16
               natural q/k (row-attention S then runs in bf16;
               -128 more matmuls/batch).
    variant 4: like 3, plus V/Acol/AcolT double-buffered across batches so
               batch b+1's projection phase (PE) can stream while batch b's
               row attention still reads V.
    variant 5: variant 2 + early transposes: exp quarters are transposed
               unnormalized as soon as they exist, and the 1/rowsum lands on
               the PV output (per-partition ACT scale) instead of on P~;
               PV and the col output use separate PSUM tiles.
    variant 6: variant 5 + the double-buffering of variant 4.
    variant 7: variant 6 + 3-deep prow pool.
    """
    FEAT = {
        1: set(),
        2: {"vt_dma"},
        3: {"vt_dma", "qkt_dma"},
        4: {"vt_dma", "qkt_dma", "dbuf"},
        5: {"vt_dma", "early_t"},
        6: {"vt_dma", "early_t", "dbuf"},
        7: {"vt_dma", "early_t", "dbuf", "prow3"},
        8: {"vt_dma", "spread"},
        9: {"vt_dma", "early_t", "spread"},
        10: {"vt_dma", "late_norm", "act_drain"},
        11: {"vt_dma", "act_drain"},
        13: {"vt_dma", "late_norm", "pe_pt", "dve_merge"},
        14: {"vt_dma", "late_norm", "dve_merge"},
        # timing-only diagnostics (wrong results): fake / absent pt transposes
        15: {"vt_dma", "fake_t"},
        16: {"vt_dma", "no_t"},
        17: {"late_norm", "pe_pt", "dve_merge"},
        18: {"vt_dma", "late_norm", "pe_pt", "dve_merge", "prow3"},
        20: {"vt_dma", "late_norm", "pe_pt", "dve_merge", "psum2"},
        22: {"late_norm", "pe_pt", "dve_merge", "f32r_pv", "slice_xt"},
        23: {"late_norm", "pe_pt", "dve_merge", "f32r_pv", "slice_xt", "prow3"},
        26: {"vt_dma", "qkt_dma", "late_norm", "pe_pt", "dve_merge"},
        27: {"vt_dma", "qkt_dma", "late_norm", "pe_pt", "dve_merge", "spread_t"},
        28: {"vt_dma", "qkt_dma", "late_norm", "pe_pt", "dve_merge", "spread_t", "dbuf"},
        # 30: St-direct row attention: S^T = K Q^T computed in [m-part, j]
        # layout per 512-token j-group, so exp() lands directly in P^T layout
        # (no P transposes at all); row sums via a ones-column matmul that
        # reuses the PV stationary; PV normalization applied at the merge.
        30: {"vt_dma", "qkt_dma", "spread_t", "st_direct"},
        # 47: v30 + a hard all-engine barrier between batches. Every corrupt
        # run of the 30+ line had batch 0 clean (the race needs cross-batch
        # overlap); the barrier removes that surface for ~60-90us of model
        # time, keeping the ~2x real marginal-time win over variant 13.
        47: {"vt_dma", "qkt_dma", "spread_t", "st_direct", "bbar"},
        # 48: v30 with the rowsum ones-matmul widened from N=1 to N=64
        # moving columns (tiny-N matmul suspected as the HW-only corruption;
        # it is the one mechanism present in every corrupt variant and
        # absent from stable variant 13).
        48: {"vt_dma", "qkt_dma", "spread_t", "st_direct", "ones64"},
        # 50: st_direct on v13's exact queue layout (no spread_t): x and y on
        # sync, all XBAR transposes on scalar — tests whether the corruption
        # is tied to the spread_t queue assignment (e.g. transposes on SP
        # colliding with the tile framework's sem housekeeping).
        50: {"vt_dma", "qkt_dma", "st_direct"},
        # 51: st_direct with ALL XBAR transposes on the scalar queue and x/y
        # on gpsimd — the synthesis of every clean/corrupt observation is
        # that transposes issued from the SP queue are the toxin.
        51: {"vt_dma", "qkt_dma", "st_direct", "spread_t2"},
        # 52: + Qt/Kt transposes issued before Vt per chunk, and all Vt
        # transposes deferred to phase-A end: B1 needs Kt chunks ASAP while
        # Vt is only consumed by B2 — pure emission-order change on the same
        # (race-safe) scalar queue.
        52: {"vt_dma", "qkt_dma", "st_direct", "spread_t2", "vt_late"},
        # race-safe buffering variants of 51 (pool-depth only):
        53: {"vt_dma", "qkt_dma", "st_direct", "spread_t2", "prow3"},
        # 56: Vt via PE transposes (v13's proven pe_pt machinery) instead of
        # the DMA XBAR, halving the scalar transpose queue that feeds B1.
        56: {"qkt_dma", "st_direct", "spread_t2", "pe_vt"},
        54: {"vt_dma", "qkt_dma", "st_direct", "spread_t2", "slice_xt", "dbuf"},
        55: {"vt_dma", "qkt_dma", "st_direct", "spread_t2", "slice_xt", "dbuf", "prow3"},
        # 31: + phase-A rebalance: per-chunk xT slices with a single merged
        # PSUM tile + one DVE copy, V-drain on Pool, Vn/AcolT double-buffered.
        31: {"vt_dma", "qkt_dma", "spread_t", "st_direct", "xsl2", "vdrain_pool", "dbuf2"},
        # 32: restructured batch body: software-pipelined phase A (projections
        # one chunk behind transposes), qn/kn stored full so col-S runs as an
        # A-end block on only 2 PSUM slots, Qt/Kt XBAR transposes on the sync
        # queue (no WAR head-of-line block at batch boundaries), Vt/AcolT on
        # scalar at A-end, x/y DMA on gpsimd.
        32: {"v32"},
        33: {"v32", "xf32r"},
        34: {"v32", "wdirect"},
        35: {"v32", "wdirect", "xf32r"},
        # 36: v32 structure but the proven DVE stt merge (no ACT scaled copy,
        # no Pool add) — isolates the HW divergence seen in 32/35.
        36: {"v32", "dve_stt"},
        37: {"v32", "dve_stt", "wdirect", "xf32r"},
        # bisection of the v32-on-HW corruption:
        38: {"v32", "dve_stt", "prow2"},           # restore ptg double-buffer
        39: {"v32", "dve_stt", "nodbuf2"},         # drop Vn/Vt double-buffer
        40: {"v32", "dve_stt", "tps_sep"},         # v30-style separate transposes
        41: {"v32", "dve_stt", "prow2", "nodbuf2", "tps_sep"},
        # bisect from the v30 side: 42 = v30-equivalent inside the v32 body
        42: {"v32", "dve_stt", "prow2", "nodbuf2", "tps_sep", "cols_inline",
             "octmp_dve", "seq_a", "kt_scalar"},
        43: {"v32", "dve_stt", "prow2", "nodbuf2", "tps_sep", "cols_inline",
             "octmp_dve", "kt_scalar"},                       # +pipelined A
        44: {"v32", "dve_stt", "prow2", "nodbuf2", "tps_sep", "cols_inline",
             "octmp_dve", "seq_a"},                           # +Kt on sync
        45: {"v32", "dve_stt", "prow2", "nodbuf2", "tps_sep",
             "octmp_dve", "seq_a", "kt_scalar"},              # +colS-late+defer
        46: {"v32", "dve_stt", "prow2", "nodbuf2", "tps_sep", "cols_inline",
             "seq_a", "kt_scalar"},                           # +octmp on ACT
    }
    feat = FEAT[variant]
    if (nb, variant, reps) in _NC_CACHE:
        return _NC_CACHE[(nb, variant, reps)]

    import concourse.bass as bass
    import concourse.tile as tile
    from concourse import bacc, mybir
    from concourse.masks import make_identity

    f32 = mybir.dt.float32
    f32r = mybir.dt.float32r
    bf16 = mybir.dt.bfloat16
    AF = mybir.ActivationFunctionType
    AX = mybir.AxisListType

    nc = bacc.Bacc("TRN2")
    xdt = f32r if "xf32r" in feat else f32
    wdt = f32r if "wdirect" in feat else f32
    x = nc.dram_tensor("x", [nb, N, E], xdt, kind="ExternalInput")
    w = nc.dram_tensor("w", [E, 3 * E], wdt, kind="ExternalInput")
    bvec = nc.dram_tensor("b", [3 * E], f32, kind="ExternalInput")
    y = nc.dram_tensor("y", [nb, N, E], f32, kind="ExternalOutput")

    with tile.TileContext(nc) as tc:
        with (
            tc.tile_pool(name="const", bufs=1) as constp,
            tc.tile_pool(name="xn", bufs=2 if ("prow3" in feat or "slice_xt" in feat) else 3) as xnp,
            tc.tile_pool(name="big", bufs=1) as bigp,
            tc.tile_pool(name="xsl", bufs=2) as xslp,
            tc.tile_pool(name="qkn", bufs=8 if "st_direct" in feat else 3) as qknp,
            tc.tile_pool(name="prow", bufs=3 if "prow3" in feat else (2 if ("prow2" in feat or "v32" not in feat) else 1)) as prowp,
            tc.tile_pool(name="stat", bufs=3) as statp,
            tc.tile_pool(name="outp", bufs=2) as outpp,
            tc.tile_pool(name="ps_proj", bufs=2, space="PSUM") as ps_proj,
            tc.tile_pool(name="ps_sc", bufs=4, space="PSUM") as ps_sc,
            tc.tile_pool(name="ps_s", bufs=2, space="PSUM") as ps_s,
        ):
            # ---------------- constants ----------------
            # W lands as float32r (rounded by the DVE copy) so fp32r matmuls
            # accept it; staged through the small xn pool to save SBUF.
            W = constp.tile([128, ED, 3 * E], f32r)
            wv = w[:].rearrange("(k p) m -> p k m", p=128)
            if "wdirect" in feat:
                # f32r is fp32 bits: DMA W straight in, no staging copies.
                for k in range(ED):
                    nc.sync.dma_start(W[:, k, :], wv[:, k, :])
            else:
                for k in range(ED):
                    for c in range(3):
                        wst = xnp.tile([128, E], f32, tag="xn", name=f"wst{k}_{c}")
                        nc.sync.dma_start(wst, wv[:, k, c * E : (c + 1) * E])
                        nc.vector.tensor_copy(W[:, k, c * E : (c + 1) * E], wst)

            # bias broadcast across partitions (for [n-part, e] layouts)
            b3 = bvec[:].rearrange("(c m) -> c m", m=E)
            bb = constp.tile([128, 3, E], bf16)
            nc.gpsimd.dma_start(
                bb, bass.AP(tensor=b3.tensor, offset=b3.offset, ap=[[0, 128]] + list(b3.ap))
            )
            # bias per partition (for [e-part, n] layouts): bpart[p, c] = b[c*128+p]
            bpart = constp.tile([128, 3 * ED], f32)
            nc.gpsimd.dma_start(bpart, bvec[:].rearrange("(c p) -> p c", p=128))

            ident = constp.tile([128, 128], f32)
            make_identity(nc, ident)
            identB = constp.tile([128, 128], bf16)
            make_identity(nc, identB)
            ones = constp.tile([128, 1], bf16)
            nc.gpsimd.memset(ones, 1.0)
            ones64 = constp.tile([128, 64], bf16)
            nc.gpsimd.memset(ones64, 1.0)
            if "xf32r" in feat:
                # gpsimd memset can't target f32r (invalid ISA) -> copy from
                # the f32 identity instead.
                identX = constp.tile([128, 128], f32r)
                nc.vector.tensor_copy(identX, ident)
            else:
                identX = ident

            def batch_body_v32():
              for b in range(nb):
                Qt = bigp.tile([128, ED, N], bf16, tag="Qt")
                Kt = bigp.tile([128, ED, N], bf16, tag="Kt")
                qf = bigp.tile([128, NE, E], bf16, tag="qf")
                kf = bigp.tile([128, NE, E], bf16, tag="kf")
                # Vn/Vt double-buffered: their writers would otherwise WAR
                # against the previous batch's B2 reads and head-of-line-block
                # the DVE / transpose queues right at the batch boundary.
                vbufs = 1 if "nodbuf2" in feat else 2
                Vn = bigp.tile([128, NE, E], bf16, tag="Vn", bufs=vbufs)
                Vt = bigp.tile([128, ED, N], bf16, tag="Vt", bufs=vbufs)
                Acol = bigp.tile([128, ED, E], bf16, tag="Acol")
                AcolT = bigp.tile([128, ED, E], bf16, tag="AcolT")

                # ---- phase A: chunk pipeline (projections one chunk behind
                # the PE transposes so the PSUM->SBUF copy is off the PE path)
                def emit_load(j):
                    xn = xnp.tile([128, E], xdt, tag="xn")
                    nc.gpsimd.dma_start(xn, x[b, j * 128 : (j + 1) * 128, :])
                    xsl = xslp.tile([128, ED, 128], f32r, tag="xsl", name=f"xsl{b}_{j}")
                    if "tps_sep" in feat:
                        for k in range(ED):
                            tps = ps_proj.tile([128, 128], xdt, tag="ps", name=f"tps{b}_{j}_{k}")
                            nc.tensor.transpose(
                                tps, xn[:, k * 128 : (k + 1) * 128], identX
                            )
                            nc.vector.tensor_copy(xsl[:, k, :], tps)
                    else:
                        tpsb = ps_proj.tile([128, E], xdt, tag="ps")
                        for k in range(ED):
                            nc.tensor.transpose(
                                tpsb[:, k * 128 : (k + 1) * 128],
                                xn[:, k * 128 : (k + 1) * 128],
                                identX,
                            )
                        nc.vector.tensor_copy(xsl.rearrange("p a b -> p (a b)"), tpsb)
                    return xsl

                def emit_proj(j, xsl):
                    jsl = slice(j * 128, (j + 1) * 128)
                    for dst, ci, deng in (
                        (qf[:, j, :], 0, nc.vector),
                        (kf[:, j, :], 1, nc.vector),
                        (Vn[:, j, :], 2, nc.vector),
                    ):
                        pp = ps_proj.tile([128, E], f32, tag="ps")
                        for k in range(ED):
                            nc.tensor.matmul(
                                pp,
                                xsl[:, k, :],
                                W[:, k, ci * E : (ci + 1) * E],
                                start=(k == 0),
                                stop=(k == ED - 1),
                            )
                        deng.tensor_add(dst, pp, bb[:, ci, :])
                    nc.sync.dma_start_transpose(Qt[:, :, jsl], qf[:, j, :])
                    kteng = nc.scalar if "kt_scalar" in feat else nc.sync
                    kteng.dma_start_transpose(Kt[:, :, jsl], kf[:, j, :])
                    vteng = (
                        (nc.sync if j % 2 else nc.scalar)
                        if "kt_scalar" in feat
                        else nc.scalar
                    )
                    vteng.dma_start_transpose(Vt[:, :, jsl], Vn[:, j, :])

                if "seq_a" in feat:
                    for j in range(NE):
                        emit_proj(j, emit_load(j))
                else:
                    prev = None
                    for j in range(NE):
                        xsl_j = emit_load(j)
                        if prev is not None:
                            emit_proj(*prev)
                        prev = (j, xsl_j)
                    emit_proj(*prev)

                # ---- col-S as an A-end block: 2 PSUM slots at a time, so the
                # previous batch's B2 po/oc can keep cycling the other 2.
                cstat = statp.tile([128, 3 * ED], f32, tag="cstat")

                def emit_colS_half(half):
                    cs = [
                        ps_sc.tile([128, E], f32, tag="scps", name=f"cs{b}_{half}_{i2}")
                        for i2 in range(2)
                    ]
                    for c in range(NE):
                        for i2 in range(2):
                            i = half * 2 + i2
                            nc.tensor.matmul(
                                cs[i2],
                                qf[:, c, i * 128 : (i + 1) * 128],
                                kf[:, c, :],
                                start=(c == 0),
                                stop=(c == NE - 1),
                            )
                    return cs

                def emit_colsm_half(half, cs):
                    for i2 in range(2):
                        i = half * 2 + i2
                        nm = cstat[:, 2 * ED + i : 2 * ED + i + 1]
                        nc.vector.reduce_max(nm, cs[i2], axis=AX.X, negate=True)
                        nc.vector.tensor_scalar_mul(nm, nm, SCALE)
                        nc.scalar.activation(
                            out=Acol[:, i, :],
                            in_=cs[i2],
                            func=AF.Exp,
                            scale=SCALE,
                            bias=nm,
                            accum_out=cstat[:, i : i + 1],
                        )

                def emit_colsm_finish():
                    nc.vector.reciprocal(cstat[:, ED : 2 * ED], cstat[:, 0:ED])
                    for i in range(ED):
                        nc.vector.tensor_scalar_mul(
                            Acol[:, i, :], Acol[:, i, :], cstat[:, ED + i : ED + i + 1]
                        )
                        nc.scalar.dma_start_transpose(
                            AcolT[:, :, i * 128 : (i + 1) * 128], Acol[:, i, :]
                        )

                if "cols_inline" in feat:
                    csall = [
                        ps_sc.tile([128, E], f32, tag="scps", name=f"cs{b}_{i}")
                        for i in range(ED)
                    ]
                    for c in range(NE):
                        for i in range(ED):
                            nc.tensor.matmul(
                                csall[i],
                                qf[:, c, i * 128 : (i + 1) * 128],
                                kf[:, c, :],
                                start=(c == 0),
                                stop=(c == NE - 1),
                            )
                    emit_colsm_half(0, csall[0:2])
                    emit_colsm_half(1, csall[2:4])
                    emit_colsm_finish()
                    cs1 = None
                else:
                    cs0 = emit_colS_half(0)
                    emit_colsm_half(0, cs0)
                    cs1 = emit_colS_half(1)
                # A2 for the second half is deferred into B1 g0 (below) so the
                # first St exps aren't queued behind the col-softmax on ACT.

                # ---- phase B: identical structure to st_direct
                for g in range(4):
                    gsl = slice(g * 512, (g + 1) * 512)
                    ptg = prowp.tile([128, NE, 512], bf16, tag="ptg")
                    for m in range(NE):
                        if g == 0 and m == 4 and cs1 is not None:
                            # deferred col-softmax half 1: its ACT exps queue
                            # behind B1's first St exps instead of ahead.
                            emit_colsm_half(1, cs1)
                            emit_colsm_finish()
                        sps = ps_s.tile([128, 512], f32, tag="s")
                        for k in range(ED):
                            nc.tensor.matmul(
                                sps,
                                Kt[:, k, m * 128 : (m + 1) * 128],
                                Qt[:, k, gsl],
                                start=(k == 0),
                                stop=(k == ED - 1),
                            )
                        nc.scalar.activation(
                            out=ptg[:, m, :], in_=sps, func=AF.Exp, scale=SCALE
                        )
                    ocs = {}
                    def emit_oc(j):
                        oc = ps_sc.tile([128, E], f32, tag="scps", name=f"oc{b}_{j}")
                        for c in range(ED):
                            nc.tensor.matmul(
                                oc,
                                Vt[:, c, j * 128 : (j + 1) * 128],
                                AcolT[:, c, :],
                                start=(c == 0),
                                stop=(c == ED - 1),
                            )
                        ocs[j] = oc
                    emit_oc(g * 4)
                    emit_oc(g * 4 + 1)
                    for jj in range(4):
                        j = g * 4 + jj
                        jpart = slice(jj * 128, (jj + 1) * 128)
                        po = ps_sc.tile([128, E], f32, tag="scps")
                        rs = ps_s.tile([128, 8], f32, tag="s", name=f"rs{b}_{j}")
                        for m in range(NE):
                            nc.tensor.matmul(
                                po,
                                ptg[:, m, jpart],
                                Vn[:, m, :],
                                start=(m == 0),
                                stop=(m == NE - 1),
                            )
                            nc.tensor.matmul(
                                rs[:, 0:1],
                                ptg[:, m, jpart],
                                ones,
                                start=(m == 0),
                                stop=(m == NE - 1),
                            )
                        rstat = statp.tile([128, 8], f32, tag="rstat")
                        nc.vector.reciprocal(rstat[:, 5:6], rs[:, 0:1])
                        if jj < 2:
                            emit_oc(g * 4 + 2 + jj)
                        ot = outpp.tile([128, E], f32, tag="ot")
                        octmp = outpp.tile([128, E], f32, tag="octmp")
                        if "dve_stt" in feat:
                            if "octmp_dve" in feat:
                                nc.vector.tensor_copy(octmp, ocs.pop(j))
                            else:
                                nc.scalar.activation(
                                    out=octmp, in_=ocs.pop(j), func=AF.Copy
                                )
                            nc.vector.scalar_tensor_tensor(
                                ot,
                                po,
                                rstat[:, 5:6],
                                octmp,
                                op0=mybir.AluOpType.mult,
                                op1=mybir.AluOpType.add,
                            )
                        else:
                            potmp = outpp.tile([128, E], f32, tag="potmp")
                            # merge off DVE entirely: ACT drains oc and the
                            # 1/rowsum-scaled po (per-partition scale), Pool
                            # does the SBUF-only add.
                            nc.scalar.activation(
                                out=octmp, in_=ocs.pop(j), func=AF.Copy
                            )
                            nc.scalar.activation(
                                out=potmp, in_=po, func=AF.Copy, scale=rstat[:, 5:6]
                            )
                            nc.gpsimd.tensor_add(ot, potmp, octmp)
                        nc.gpsimd.dma_start(y[b, j * 128 : (j + 1) * 128, :], ot)

            def batch_body():
              if "v32" in feat:
                  return batch_body_v32()
              for b in range(nb):
                qkt_dt = bf16 if "qkt_dma" in feat else f32r
                vn_dt = f32r if "f32r_pv" in feat else bf16
                dbufs = 2 if "dbuf" in feat else 1
                dbufs2 = 2 if "dbuf2" in feat else dbufs
                slice_xt = "slice_xt" in feat or "xsl2" in feat
                if not slice_xt:
                    xT = bigp.tile([128, ED, N], f32r, tag="xT")
                Qt = bigp.tile([128, ED, N], qkt_dt, tag="Qt")
                Kt = bigp.tile([128, ED, N], qkt_dt, tag="Kt")
                Vn = bigp.tile([128, NE, E], vn_dt, tag="Vn", bufs=dbufs2)
                Vt = bigp.tile([128, ED, N], bf16, tag="Vt")
                Acol = bigp.tile([128, ED, E], bf16, tag="Acol", bufs=dbufs)
                AcolT = bigp.tile([128, ED, E], bf16, tag="AcolT", bufs=dbufs2)
                scol_ps = [
                    ps_sc.tile([128, E], f32, tag="scps", name=f"scol{b}_{i}")
                    for i in range(ED)
                ]

                # ---- phase A: load x, build xT, projections, col-S accumulation
                for j in range(NE):
                    s_idx, jj = j // ED, j % ED
                    if "xsl2" in feat:
                        xT = xslp.tile(
                            [128, ED, 128], f32r, tag="xsl", name=f"xsl{b}_{j}"
                        )
                        xoff, roff = 0, 0
                    elif slice_xt:
                        if jj == 0:
                            xT = xslp.tile(
                                [128, ED, ED * 128], f32r, tag="xsl", name=f"xsl{b}_{s_idx}"
                            )
                        xoff, roff = jj * 128, 0
                    else:
                        xoff, roff = j * 128, s_idx * 512
                    xn = xnp.tile([128, E], f32, tag="xn")
                    xeng = (
                        nc.gpsimd
                        if ("spread_t" in feat or "spread_t2" in feat)
                        else nc.sync
                    )
                    xeng.dma_start(xn, x[b, j * 128 : (j + 1) * 128, :])
                    if "xsl2" in feat:
                        # all 4 transposes land in one PSUM tile -> one copy
                        tpsb = ps_proj.tile([128, E], f32, tag="ps")
                        for k in range(ED):
                            nc.tensor.transpose(
                                tpsb[:, k * 128 : (k + 1) * 128],
                                xn[:, k * 128 : (k + 1) * 128],
                                ident,
                            )
                        nc.vector.tensor_copy(
                            xT.rearrange("p a b -> p (a b)"), tpsb
                        )
                    else:
                        for k in range(ED):
                            tps = ps_proj.tile([128, 128], f32, tag="ps")
                            nc.tensor.transpose(tps, xn[:, k * 128 : (k + 1) * 128], ident)
                            nc.vector.tensor_copy(xT[:, k, xoff : xoff + 128], tps)

                    # natural-layout q, k, v for this token chunk
                    qn = qknp.tile([128, E], bf16, tag="qn")
                    kn = qknp.tile([128, E], bf16, tag="kn")
                    for dst, ci in ((qn, 0), (kn, 1), (Vn[:, j, :], 2)):
                        pp = ps_proj.tile([128, E], f32, tag="ps")
                        for k in range(ED):
                            nc.tensor.matmul(
                                pp,
                                xT[:, k, xoff : xoff + 128],
                                W[:, k, ci * E : (ci + 1) * E],
                                start=(k == 0),
                                stop=(k == ED - 1),
                            )
                        deng = (
                            nc.gpsimd
                            if (ci == 2 and "vdrain_pool" in feat)
                            else nc.vector
                        )
                        deng.tensor_add(dst, pp, bb[:, ci, :])

                    # col-attention S accumulation: S_col[d,e] += q_j.T @ k_j
                    for i in range(ED):
                        nc.tensor.matmul(
                            scol_ps[i],
                            qn[:, i * 128 : (i + 1) * 128],
                            kn,
                            start=(j == 0),
                            stop=(j == NE - 1),
                        )

                    # bf16 transposed layouts via the DMA XBAR (free wrt PE)
                    jsl = slice(j * 128, (j + 1) * 128)
                    if "pe_vt" in feat:
                        for kq in range(ED):
                            vps = ps_proj.tile(
                                [128, 128], bf16, tag="ps", name=f"vps{b}_{j}_{kq}"
                            )
                            nc.tensor.transpose(
                                vps, Vn[:, j, kq * 128 : (kq + 1) * 128], identB
                            )
                            nc.vector.tensor_copy(Vt[:, kq, jsl], vps)
                    if "qkt_dma" in feat and "vt_late" in feat:
                        # B1 consumes Kt/Qt first: put them at the queue head
                        nc.scalar.dma_start_transpose(Qt[:, :, jsl], qn)
                        nc.scalar.dma_start_transpose(Kt[:, :, jsl], kn)
                    if "vt_dma" in feat and "vt_late" not in feat:
                        veng = (
                            (nc.sync if j % 2 else nc.scalar)
                            if "spread_t" in feat
                            else nc.scalar
                        )
                        veng.dma_start_transpose(Vt[:, :, jsl], Vn[:, j, :])
                    if "qkt_dma" in feat and "vt_late" not in feat:
                        qeng = nc.sync if "spread_t" in feat else nc.scalar
                        qeng.dma_start_transpose(Qt[:, :, jsl], qn)
                        nc.scalar.dma_start_transpose(Kt[:, :, jsl], kn)

                    # transposed-layout projections, one 512-token slice at a time
                    if "qkt_dma" in feat:
                        tproj = ()
                    elif "vt_dma" in feat:
                        tproj = ((Qt, 0), (Kt, 1))
                    else:
                        tproj = ((Qt, 0), (Kt, 1), (Vt, 2))
                    if j % ED == ED - 1 and tproj:
                        sl = slice(s_idx * 512, (s_idx + 1) * 512)
                        for dst, ci in tproj:
                            for i in range(ED):
                                pp = ps_proj.tile([128, E], f32, tag="ps")
                                for k in range(ED):
                                    nc.tensor.matmul(
                                        pp,
                                        W[:, k, ci * E + i * 128 : ci * E + (i + 1) * 128],
                                        xT[:, k, roff : roff + 512],
                                        start=(k == 0),
                                        stop=(k == ED - 1),
                                    )
                                if "act_drain" in feat:
                                    nc.scalar.activation(
                                        out=dst[:, i, sl],
                                        in_=pp,
                                        func=AF.Identity,
                                        bias=bpart[:, ci * ED + i : ci * ED + i + 1],
                                    )
                                else:
                                    nc.vector.tensor_scalar_add(
                                        dst[:, i, sl], pp, bpart[:, ci * ED + i : ci * ED + i + 1]
                                    )

                # ---- phase A2: col softmax + transpose of A
                # col logits are O(+-600): subtract the per-row max (as an ACT
                # bias of -max*SCALE) before exp, unlike the row path.
                cstat = statp.tile([128, 3 * ED], f32, tag="cstat")
                for i in range(ED):
                    nm = cstat[:, 2 * ED + i : 2 * ED + i + 1]
                    nc.vector.reduce_max(nm, scol_ps[i], axis=AX.X, negate=True)
                    nc.vector.tensor_scalar_mul(nm, nm, SCALE)
                    nc.scalar.activation(
                        out=Acol[:, i, :],
                        in_=scol_ps[i],
                        func=AF.Exp,
                        scale=SCALE,
                        bias=nm,
                        accum_out=cstat[:, i : i + 1],
                    )
                nc.vector.reciprocal(cstat[:, ED : 2 * ED], cstat[:, 0:ED])
                for i in range(ED):
                    nc.vector.tensor_scalar_mul(
                        Acol[:, i, :], Acol[:, i, :], cstat[:, ED + i : ED + i + 1]
                    )
                    nc.scalar.dma_start_transpose(
                        AcolT[:, :, i * 128 : (i + 1) * 128], Acol[:, i, :]
                    )

                if "vt_late" in feat:
                    for jv in range(NE):
                        nc.scalar.dma_start_transpose(
                            Vt[:, :, jv * 128 : (jv + 1) * 128], Vn[:, jv, :]
                        )

                # ---- phase B (st_direct): S^T = K Q^T per 512-token j-group;
                # exp() output IS P~^T (no transposes); row sums via a ones
                # column reusing the PV stationary; 1/rowsum at the merge.
                if "st_direct" in feat:
                    for g in range(4):
                        gsl = slice(g * 512, (g + 1) * 512)
                        ptg = prowp.tile([128, NE, 512], bf16, tag="ptg")
                        for m in range(NE):
                            sps = ps_s.tile([128, 512], f32, tag="s")
                            for k in range(ED):
                                nc.tensor.matmul(
                                    sps,
                                    Kt[:, k, m * 128 : (m + 1) * 128],
                                    Qt[:, k, gsl],
                                    start=(k == 0),
                                    stop=(k == ED - 1),
                                )
                            nc.scalar.activation(
                                out=ptg[:, m, :], in_=sps, func=AF.Exp, scale=SCALE
                            )
                        # col-attention output for this group's 4 chunks: fills
                        # the PE while the last exp() quarters drain.
                        ocs = {}
                        def emit_oc(j):
                            oc = ps_sc.tile(
                                [128, E], f32, tag="scps", name=f"oc{b}_{j}"
                            )
                            for c in range(ED):
                                nc.tensor.matmul(
                                    oc,
                                    Vt[:, c, j * 128 : (j + 1) * 128],
                                    AcolT[:, c, :],
                                    start=(c == 0),
                                    stop=(c == ED - 1),
                                )
                            ocs[j] = oc
                        emit_oc(g * 4)
                        emit_oc(g * 4 + 1)
                        for jj in range(4):
                            j = g * 4 + jj
                            jpart = slice(jj * 128, (jj + 1) * 128)
                            po = ps_sc.tile([128, E], f32, tag="scps")
                            rs = ps_s.tile(
                                [128, 64 if "ones64" in feat else 8], f32,
                                tag="s", name=f"rs{b}_{j}",
                            )
                            onesv = ones64 if "ones64" in feat else ones
                            rsl = slice(0, 64) if "ones64" in feat else slice(0, 1)
                            for m in range(NE):
                                nc.tensor.matmul(
                                    po,
                                    ptg[:, m, jpart],
                                    Vn[:, m, :],
                                    start=(m == 0),
                                    stop=(m == NE - 1),
                                )
                                nc.tensor.matmul(
                                    rs[:, rsl],
                                    ptg[:, m, jpart],
                                    onesv,
                                    start=(m == 0),
                                    stop=(m == NE - 1),
                                )
                            rstat = statp.tile([128, 8], f32, tag="rstat")
                            nc.vector.reciprocal(rstat[:, 5:6], rs[:, 0:1])
                            if jj < 2:
                                emit_oc(g * 4 + 2 + jj)
                            ot = outpp.tile([128, E], f32, tag="ot")
                            octmp = outpp.tile([128, E], f32, tag="octmp")
                            nc.vector.tensor_copy(octmp, ocs.pop(j))
                            nc.vector.scalar_tensor_tensor(
                                ot,
                                po,
                                rstat[:, 5:6],
                                octmp,
                                op0=mybir.AluOpType.mult,
                                op1=mybir.AluOpType.add,
                            )
                            yeng2 = (
                                nc.gpsimd
                                if ("spread_t" in feat or "spread_t2" in feat)
                                else nc.sync
                            )
                            yeng2.dma_start(
                                y[b, j * 128 : (j + 1) * 128, :], ot
                            )
                    if "bbar" in feat and b < nb - 1:
                        # empty 1-iteration hardware loop = supported
                        # all-engine barrier between batches
                        with tc.For_i(0, 1, 1):
                            pass
                    continue

                # ---- phase B: row attention + merged output, per token chunk
                early_t = "early_t" in feat
                late_norm = "late_norm" in feat
                spread = "spread" in feat
                for j in range(NE):
                    teng = (nc.sync if j % 2 else nc.scalar) if spread else nc.scalar
                    yeng = (nc.scalar if j % 2 else nc.sync) if spread else nc.sync
                    if "spread_t" in feat:
                        yeng = nc.gpsimd
                    pt = prowp.tile([128, N], bf16, tag="pt")
                    ptT = prowp.tile(
                        [128, NE, 128], f32r if "f32r_pv" in feat else bf16, tag="ptT"
                    )
                    rstat = statp.tile([128, 8], f32, tag="rstat")
                    for q in range(4):
                        sps = ps_s.tile([128, 512], f32, tag="s")
                        for k in range(ED):
                            nc.tensor.matmul(
                                sps,
                                Qt[:, k, j * 128 : (j + 1) * 128],
                                Kt[:, k, q * 512 : (q + 1) * 512],
                                start=(k == 0),
                                stop=(k == ED - 1),
                            )
                        nc.scalar.activation(
                            out=pt[:, q * 512 : (q + 1) * 512],
                            in_=sps,
                            func=AF.Exp,
                            scale=SCALE,
                            accum_out=rstat[:, q : q + 1],
                        )
                        if early_t:
                            # transpose the unnormalized quarter right away;
                            # 1/rowsum is applied to the PV output instead
                            teng.dma_start_transpose(
                                ptT[:, 4 * q : 4 * q + 4, :],
                                pt[:, q * 512 : (q + 1) * 512],
                            )
                        if "pe_pt" in feat:
                            for t in range(4):
                                m = 4 * q + t
                                psB = ps_proj.tile(
                                    [128, 128], bf16, tag="ps", name=f"psB{b}_{j}_{m}"
                                )
                                nc.tensor.transpose(
                                    psB, pt[:, m * 128 : (m + 1) * 128], identB
                                )
                                nc.vector.tensor_copy(ptT[:, m, :], psB)
                    nc.vector.reduce_sum(rstat[:, 4:5], rstat[:, 0:4], axis=AX.X)
                    nc.vector.reciprocal(rstat[:, 5:6], rstat[:, 4:5])
                    if late_norm:
                        if "pe_pt" not in feat:
                            teng.dma_start_transpose(ptT, pt)
                    elif not early_t:
                        nc.vector.tensor_scalar_mul(pt, pt, rstat[:, 5:6])
                        if "fake_t" in feat:
                            teng.dma_start(ptT.rearrange("p a b -> p (a b)"), pt)
                        elif "no_t" in feat:
                            nc.vector.tensor_copy(ptT[:, 0, :], pt[:, :128])
                        else:
                            teng.dma_start_transpose(ptT, pt)

                    po = ps_sc.tile([128, E], f32, tag="scps")
                    for m in range(NE):
                        nc.tensor.matmul(
                            po,
                            ptT[:, m, :],
                            Vn[:, m, :],
                            start=(m == 0),
                            stop=((early_t or late_norm) and m == NE - 1),
                        )
                    ot = outpp.tile([128, E], f32, tag="ot")
                    if early_t or late_norm:
                        oc = ps_sc.tile([128, E], f32, tag="scps")
                        for c in range(ED):
                            nc.tensor.matmul(
                                oc,
                                Vt[:, c, j * 128 : (j + 1) * 128],
                                AcolT[:, c, :],
                                start=(c == 0),
                                stop=(c == ED - 1),
                            )
                        if "dve_merge" in feat:
                            if "psum2" in feat:
                                nc.vector.scalar_tensor_tensor(
                                    ot,
                                    po,
                                    rstat[:, 5:6],
                                    oc,
                                    op0=mybir.AluOpType.mult,
                                    op1=mybir.AluOpType.add,
                                )
                            else:
                                octmp = outpp.tile([128, E], f32, tag="octmp")
                                nc.vector.tensor_copy(octmp, oc)
                                nc.vector.scalar_tensor_tensor(
                                    ot,
                                    po,
                                    rstat[:, 5:6],
                                    octmp,
                                    op0=mybir.AluOpType.mult,
                                    op1=mybir.AluOpType.add,
                                )
                        else:
                            nc.scalar.activation(
                                out=ot, in_=po, func=AF.Copy, scale=rstat[:, 5:6]
                            )
                            nc.vector.tensor_add(ot, ot, oc)
                    else:
                        for c in range(ED):
                            nc.tensor.matmul(
                                po,
                                Vt[:, c, j * 128 : (j + 1) * 128],
                                AcolT[:, c, :],
                                start=False,
                                stop=(c == ED - 1),
                            )
                        nc.vector.tensor_copy(ot, po)
                    yeng.dma_start(y[b, j * 128 : (j + 1) * 128, :], ot)

            if reps == 1:
                batch_body()
            else:
                with tc.For_i(0, reps, 1):
                    batch_body()

    nc.compile()
    _NC_CACHE[(nb, variant, reps)] = nc
    return nc


def make_in_maps(x, w_qkv, b_qkv):
    xs = np.ascontiguousarray(np.asarray(x, dtype=np.float32)).reshape(B, N, E)
    w = np.ascontiguousarray(np.asarray(w_qkv, dtype=np.float32))
    bq = np.ascontiguousarray(np.asarray(b_qkv, dtype=np.float32))
    return [
        {"x": np.ascontiguousarray(xs[c * NB : (c + 1) * NB]), "w": w, "b": bq}
        for c in range(NCORES)
    ]


BEST_VARIANT = 51

_DISPATCH_CACHE = {}


def _get_dispatch(variant=BEST_VARIANT):
    """Build (once) the persistent jitted 8-core dispatcher for the kernel.

    Mirrors concourse.bass2jax.run_bass_via_pjrt's shard_map structure (the
    bass_exec custom call only tolerates parameters as operands), but:
      - the jit + device buffers are cached at module level so repeat
        kernel() calls skip retrace/recompile;
      - w/b are replicated via PartitionSpec() instead of 8x host-concat;
      - the ExternalOutput zero-operand is NOT donated: the kernel writes
        every element of y, so one cached device-resident zeros buffer is
        reused across calls (no 128MB host->device zero upload per call).
    """
    if variant in _DISPATCH_CACHE:
        return _DISPATCH_CACHE[variant]

    import jax
    from jax.experimental.shard_map import shard_map
    from jax.sharding import Mesh, NamedSharding, PartitionSpec

    from concourse.bass2jax import (
        _bass_exec_p,
        install_neuronx_cc_hook,
        partition_id_tensor,
    )

    install_neuronx_cc_hook()
    nc = build_nc(NB, variant)
    pname = nc.partition_id_tensor.name if nc.partition_id_tensor else None
    in_names = ("x", "w", "b", "y") + ((pname,) if pname else ())
    out_avals = (jax.core.ShapedArray((NB, N, E), np.float32),)

    def _body(x_, w_, b_, z_):
        operands = [x_, w_, b_, z_]
        if pname is not None:
            operands.append(partition_id_tensor())
        outs = _bass_exec_p.bind(
            *operands,
            out_avals=out_avals,
            in_names=in_names,
            out_names=("y",),
            lowering_input_output_aliases=(),
            sim_require_finite=True,
            sim_require_nnan=True,
            nc=nc,
        )
        return outs[0]

    devices = jax.devices()[:NCORES]
    mesh = Mesh(np.asarray(devices), ("core",))
    P = PartitionSpec
    fn = jax.jit(
        shard_map(
            _body,
            mesh=mesh,
            in_specs=(P("core"), P(), P(), P("core")),
            out_specs=P("core"),
            check_rep=False,
        ),
        keep_unused=True,
    )
    shx = NamedSharding(mesh, P("core"))
    shr = NamedSharding(mesh, P())
    zeros = jax.device_put(np.zeros((B, N, E), np.float32), shx)
    d = (nc, fn, shx, shr, zeros)
    _DISPATCH_CACHE[variant] = d
    return d


def kernel(x, w_qkv, b_qkv):
    import jax

    nc, fn, shx, shr, zeros = _get_dispatch()
    xs = np.ascontiguousarray(np.asarray(x, dtype=np.float32)).reshape(B, N, E)
    xd = jax.device_put(xs, shx)
    wd = jax.device_put(np.ascontiguousarray(np.asarray(w_qkv, np.float32)), shr)
    bd = jax.device_put(np.ascontiguousarray(np.asarray(b_qkv, np.float32)), shr)
    return np.asarray(fn(xd, wd, bd, zeros))


def get_dispatcher(x, w_qkv, b_qkv, variant=BEST_VARIANT):
    """For profiling: returns (nc, run_fn) where run_fn() performs exactly
    one on-device dispatch with device-resident inputs and blocks."""
    import jax

    nc, fn, shx, shr, zeros = _get_dispatch(variant)
    xs = np.ascontiguousarray(np.asarray(x, dtype=np.float32)).reshape(B, N, E)
    xd = jax.device_put(xs, shx)
    wd = jax.device_put(np.ascontiguousarray(np.asarray(w_qkv, np.float32)), shr)
    bd = jax.device_put(np.ascontiguousarray(np.asarray(b_qkv, np.float32)), shr)

    def run_fn():
        return jax.block_until_ready(fn(xd, wd, bd, zeros))

    return nc, run_fn



# revision 20
# speedup vs baseline: 72.5762x; 1.1413x over previous
"""AxialSelfAttention Trainium2 Bass kernel.

Reference computation (per batch b):
    xs  = x[b] reshaped [N=2048, E=512]
    qkv = xs @ W + bias                      # [N, 3E]
    q, k, v = split(qkv)
    row:  P = softmax(q @ k.T / sqrt(E));  out_row = P @ v
    col:  A = softmax(q.T @ k / sqrt(E));  out_col = v @ A.T
    out = out_row + out_col                  # [N, E]

Sharding: data-parallel over batch B=32 across 8 cores (4 batches/core).

Shipped variant: 69 = v51's St-direct algorithm with this session's
NTFF-profile-driven fixes on top:
  - ONE [128,1536] XBAR transpose per chunk writes Q^T|K^T|V^T into a
    merged QKVt tile (DMA_TRANSPOSE engine-hold is ~1.2us FIXED
    regardless of size; v51 paid 3 holds/chunk and its transpose queue
    ran ~3 chunks behind the projections, stalling hoisted B1 matmuls).
    Natural-layout Vn is kept via one extra DVE copy per chunk.
  - bf16 projections (W bf16 direct-DMA from host, x cast to bf16
    before its PE transposes - f32 PE transposes cost 600ns vs 219ns).
  - rs rowsum PSUM tiles moved from the 2-bank ps_s pool (shared with
    St sps tiles) to ps_proj, removing a group-boundary PSUM WAR.
NTFF profile (neuron-profile HW exec time, max across 8 cores):
~0.97-1.00ms vs v51's ~1.11-1.15ms (-13%). Device-verified
BIT-DETERMINISTIC, rel err 8.1476e-3 (bf16 W/x quantization; gate
2e-2).  PE busy ~842us of ~1080us total; remaining gaps are framework
init (~14us), batch boundaries (~7us x3), and the A2 col-softmax
serial chain (~5us/batch - reduce_max is MANDATORY, scaled col logits
are +-570, exp overflows f32 without it; v66/v67 NaN'd).
Measurement notes: wall-clock dispatch over the axon tunnel is ~30-120
ms and measures ONLY the RPC (a 1-device a+1 jit costs the same), so
test.py reports the NTFF profile time. Single-window profiles drift
+-30us with DVFS throttling (throttle_active 150-250us/run); compare
variants with interleaved A/B windows in one process (ab.py).
Failed/rejected this session: tensor_tensor_reduce crashes the worker
(mesh desync) - use scalar_tensor_tensor with accum_out instead;
rs_group (replicated ones-stat rowsums, v65) was NET SLOWER because
the rowsum riders mostly hide behind PV pipelining; col_nomax NaNs;
gpsimd cannot issue XBAR transposes (HWDGE = SP/ACT only, SP is the
proven-corrupt queue).
(v51 at rel err 1.2320e-3 is the fallback if anything regresses.)

Real-device-time model, fitted on the two reps-delta measurements
(v26: 1.22ms @ 1084us-model/14033 insts; v30: ~0.95ms @ 850us/11913):
  real_per_rep ~= model_ns + 9.7ns * instruction_count
(validated: predicted 1.107ms for v51, measured 1.14ms — within 3%)
Under it the remaining screened candidates (52: flat; 54: -0.6% model;
56: -1.9% model but +973 insts) are all within noise of v51 (~1.10ms).
Real further gains need PE-work cuts (blocked: fp8 fails accuracy) or
the ~850us spread-queue schedule (blocked: SP-queue XBAR corruption).

The v50/v51 'deterministic 0.866' bug was NOT a framework issue: a
scripted edit had de-indented the y-store out of the jj-loop so only
j=3,7,11,15 were stored. Fixed; v51 then passed 5/5 bit-identical.

ROOT CAUSE of the long-chased corruption: the spread_t queue layout.
v26 (no spread) is bit-stable; v27 (= v26 + spread_t: Qt-T on the SP
queue, Vt-T alternating, x/y DMA on gpsimd) corrupts nondeterministically
up to 5e-2. Every corrupt variant (27-48) had spread_t; both clean
lines (13, 26) do not. Suspected mechanism: XBAR-transpose issues on the
SP queue colliding with the tile framework's semaphore housekeeping
(BassTileRelease runs on SP), or the gpsimd SWDGE x/y path.

st_direct follow-ups (v50/v51): moving the Qt/Vt transposes to the
scalar queue inside st_direct gives a DETERMINISTIC rel err 0.866 —
1536/2048 output rows all-zero (12 of 16 j-chunks never stored), same
value on device and in CoreSim (v51 restored x/y to gpsimd and it did
not change). Since it reproduces in CoreSim it is debuggable offline:
run CoreSim nb=1 on v50, find which j-chunks are unwritten and walk the
B2 store chain back (likely a framework scheduling/dep issue when all
52 XBAR transposes + 68 exps share the ACT queue). Fixing that unlocks
~968us-model st_direct on race-free queues -> est. ~1.0-1.1ms/rep.

Variant 30+ ("St-direct", see FEAT below) is the faster research line:
row attention computed transposed (St = K Q^T per 512-token j-group) so
exp() lands directly in P^T layout, Qt/Kt via DMA-XBAR transposes, row
sums via a ones-column matmul reusing the PV stationary. It models at
850us/core vs 13's 1022us and measured ~2x lower marginal device time
per rep (reps-delta: ~0.95ms vs 1.93ms), BUT it shows a nondeterministic
whole-chunk corruption on hardware (batch 0 always clean, later batches
intermittently 0.004-0.018 global rel err for the v30 family, 0.06-0.13
for the v32 family; CoreSim and TimelineSim both clean => HW-only).

Race-hunt results (all on device, single-shot kernel() trials):
  - NOT the ptg single-buffer, Vn/Vt dbuf, or merged tpsb transposes
    (v41 corrupt with all three reverted).
  - NOT the ACT-copy/Pool-add merge (v36 corrupt with DVE stt merge).
  - NOT a cross-batch-overlap-only effect: v47 (inter-batch empty-For_i
    barrier) still corrupt 4/4 trials.
  - NOT the N=1 ones-matmul rowsum alone: v48 (64-wide ones) still
    corrupt 1/5 trials (others at baseline 1.3e-3).
  - Wider qn/kn pool (8 bufs) did not help either.
  RESOLVED: v26 (qkt_dma, no spread) bit-stable across trials while
  v27 (+spread_t) corrupts to 5e-2 => the spread_t queue layout is the
  race; qkt_dma itself is innocent. Synthesis over all runs: corruption
  amplitude tracks how many XBAR transposes are issued from the SP
  queue (v32 family with Qt+Kt on SP worst; v42 with only Qt mild).
"""

import sys

for _p in ("/opt/trn_rl_repo", "/root/.axon_site/_ro/trn_rl_repo"):
    if _p not in sys.path:
        sys.path.append(_p)

import numpy as np

B, N, E = 32, 2048, 512
NCORES = 8
NB = B // NCORES  # batches per core
NE = N // 128  # 16 token chunks
ED = E // 128  # 4 feature chunks
SCALE = 1.0 / float(np.sqrt(E))

_NC_CACHE = {}


def build_nc(nb=NB, variant=13, reps=1):
    """Build (once) the single-core Bass module processing nb batches.

    variant 1: all six projection layouts via fp32r matmuls from xT.
    variant 2: like 1, but Vt comes from a bf16 DMA(XBAR)-transpose of V
               instead of its own matmul projection (-64 matmuls/batch).
    variant 3: like 2, and Qt/Kt also come from DMA-transposes of the bf16
               natural q/k (row-attention S then runs in bf16;
               -128 more matmuls/batch).
    variant 4: like 3, plus V/Acol/AcolT double-buffered across batches so
               batch b+1's projection phase (PE) can stream while batch b's
               row attention still reads V.
    variant 5: variant 2 + early transposes: exp quarters are transposed
               unnormalized as soon as they exist, and the 1/rowsum lands on
               the PV output (per-partition ACT scale) instead of on P~;
               PV and the col output use separate PSUM tiles.
    variant 6: variant 5 + the double-buffering of variant 4.
    variant 7: variant 6 + 3-deep prow pool.
    """
    FEAT = {
        1: set(),
        2: {"vt_dma"},
        3: {"vt_dma", "qkt_dma"},
        4: {"vt_dma", "qkt_dma", "dbuf"},
        5: {"vt_dma", "early_t"},
        6: {"vt_dma", "early_t", "dbuf"},
        7: {"vt_dma", "early_t", "dbuf", "prow3"},
        8: {"vt_dma", "spread"},
        9: {"vt_dma", "early_t", "spread"},
        10: {"vt_dma", "late_norm", "act_drain"},
        11: {"vt_dma", "act_drain"},
        13: {"vt_dma", "late_norm", "pe_pt", "dve_merge"},
        14: {"vt_dma", "late_norm", "dve_merge"},
        # timing-only diagnostics (wrong results): fake / absent pt transposes
        15: {"vt_dma", "fake_t"},
        16: {"vt_dma", "no_t"},
        17: {"late_norm", "pe_pt", "dve_merge"},
        18: {"vt_dma", "late_norm", "pe_pt", "dve_merge", "prow3"},
        20: {"vt_dma", "late_norm", "pe_pt", "dve_merge", "psum2"},
        22: {"late_norm", "pe_pt", "dve_merge", "f32r_pv", "slice_xt"},
        23: {"late_norm", "pe_pt", "dve_merge", "f32r_pv", "slice_xt", "prow3"},
        26: {"vt_dma", "qkt_dma", "late_norm", "pe_pt", "dve_merge"},
        27: {"vt_dma", "qkt_dma", "late_norm", "pe_pt", "dve_merge", "spread_t"},
        28: {"vt_dma", "qkt_dma", "late_norm", "pe_pt", "dve_merge", "spread_t", "dbuf"},
        # 30: St-direct row attention: S^T = K Q^T computed in [m-part, j]
        # layout per 512-token j-group, so exp() lands directly in P^T layout
        # (no P transposes at all); row sums via a ones-column matmul that
        # reuses the PV stationary; PV normalization applied at the merge.
        30: {"vt_dma", "qkt_dma", "spread_t", "st_direct"},
        # 47: v30 + a hard all-engine barrier between batches. Every corrupt
        # run of the 30+ line had batch 0 clean (the race needs cross-batch
        # overlap); the barrier removes that surface for ~60-90us of model
        # time, keeping the ~2x real marginal-time win over variant 13.
        47: {"vt_dma", "qkt_dma", "spread_t", "st_direct", "bbar"},
        # 48: v30 with the rowsum ones-matmul widened from N=1 to N=64
        # moving columns (tiny-N matmul suspected as the HW-only corruption;
        # it is the one mechanism present in every corrupt variant and
        # absent from stable variant 13).
        48: {"vt_dma", "qkt_dma", "spread_t", "st_direct", "ones64"},
        # 50: st_direct on v13's exact queue layout (no spread_t): x and y on
        # sync, all XBAR transposes on scalar — tests whether the corruption
        # is tied to the spread_t queue assignment (e.g. transposes on SP
        # colliding with the tile framework's sem housekeeping).
        50: {"vt_dma", "qkt_dma", "st_direct"},
        # 51: st_direct with ALL XBAR transposes on the scalar queue and x/y
        # on gpsimd — the synthesis of every clean/corrupt observation is
        # that transposes issued from the SP queue are the toxin.
        51: {"vt_dma", "qkt_dma", "st_direct", "spread_t2"},
        # 52: + Qt/Kt transposes issued before Vt per chunk, and all Vt
        # transposes deferred to phase-A end: B1 needs Kt chunks ASAP while
        # Vt is only consumed by B2 — pure emission-order change on the same
        # (race-safe) scalar queue.
        52: {"vt_dma", "qkt_dma", "st_direct", "spread_t2", "vt_late"},
        # race-safe buffering variants of 51 (pool-depth only):
        53: {"vt_dma", "qkt_dma", "st_direct", "spread_t2", "prow3"},
        # 56: Vt via PE transposes (v13's proven pe_pt machinery) instead of
        # the DMA XBAR, halving the scalar transpose queue that feeds B1.
        56: {"qkt_dma", "st_direct", "spread_t2", "pe_vt"},
        54: {"vt_dma", "qkt_dma", "st_direct", "spread_t2", "slice_xt", "dbuf"},
        55: {"vt_dma", "qkt_dma", "st_direct", "spread_t2", "slice_xt", "dbuf", "prow3"},
        # 61: v51 + bf16 projections: W staged to bf16, x transposed (PE) into
        # per-chunk bf16 xsl slices (one DVE copy per chunk), so the qkv
        # projection matmuls run at full bf16 PE rate instead of f32r's
        # half rate. Predicted -160us/core PE busy. (Measured: only -30us
        # PE busy — f32r matmul is nearly full rate on TRN2 — but keeps
        # -4MB SBUF and the bf16 staging; rel err 8.1e-3, bit-stable 3/3.)
        61: {"vt_dma", "qkt_dma", "st_direct", "spread_t2", "wb16"},
        # 62: v61 + queue re-spread: ALL XBAR transposes (Qt/Kt/Vt/AcolT)
        # move scalar->gpsimd, x/y DMA moves gpsimd->sync (x/y-on-sync was
        # bit-stable in v26). Scalar becomes exp-only: the trace shows PE
        # stalling 4-6us several times per batch on exps queued behind
        # ~21us/batch of SWDGE descgen on the scalar engine.
        62: {"vt_dma", "qkt_dma", "st_direct", "wb16", "tp_gpsimd"},
        # 64: v61 + three trace-driven fixes: (a) rs rowsum tiles move from
        # the 2-bank ps_s pool (shared with St sps) to ps_proj, removing the
        # St(g+1)-vs-recip(rs) PSUM WAR stall at every group boundary;
        # (b) x PE transposes run bf16 (DVE cast first) - f32 transposes
        # cost 600ns each, 154us/core; (c) W arrives bf16 from the host
        # (no staging copies at kernel start).
        64: {"vt_dma", "qkt_dma", "st_direct", "wb16", "rs_proj", "xb16t",
             "wb16d"},
        # 65: v64 + rs_group: the 256/batch one-column rowsum rider matmuls
        # (166us/core of PE issue slots) are replaced by 16 ones-stationary
        # matmuls per group into a replicated [128,512] PSUM (rowsum(q) on
        # every partition), extracted per j-chunk with a single DVE
        # tensor_tensor_reduce against the identity (diagonal pick). Same
        # products, same accumulation order => bit-identical rowsums.
        65: {"vt_dma", "qkt_dma", "st_direct", "wb16", "xb16t", "wb16d",
             "rs_group"},
        # 66: v64 + qk1t: q and k land in one contiguous [128,1024] tile and
        # ONE XBAR transpose per chunk writes both Q^T and K^T halves of a
        # merged QKt tile (the scalar transpose queue lags ~3 chunks behind
        # the projections; halving its entries amortizes the ~1.2us
        # engine-hold). + col_nomax: col logits*SCALE are O(27), far below
        # f32 exp overflow (88), so the reduce_max/bias chain in A2 (serial
        # ~3us/batch in front of B1's first exps) is dropped.
        66: {"vt_dma", "qkt_dma", "st_direct", "wb16", "rs_proj", "xb16t",
             "wb16d", "qk1t", "col_nomax"},
        67: {"vt_dma", "qkt_dma", "st_direct", "wb16", "rs_proj", "xb16t",
             "wb16d", "col_nomax"},
        68: {"vt_dma", "qkt_dma", "st_direct", "wb16", "rs_proj", "xb16t",
             "wb16d", "qk1t"},
        # 69: v68 with Vt folded into the same per-chunk transpose: ONE
        # [128,1536] qkv XBAR call per chunk (holds are ~1.2us fixed
        # regardless of size, so 3->1 calls saves ~19us/batch of scalar
        # hold). Natural-layout Vn is produced by duplicating the v bias
        # add onto the idle Pool engine.
        69: {"qkt_dma", "st_direct", "wb16", "rs_proj", "xb16t", "wb16d",
             "qkv1t"},
        # 31: + phase-A rebalance: per-chunk xT slices with a single merged
        # PSUM tile + one DVE copy, V-drain on Pool, Vn/AcolT double-buffered.
        31: {"vt_dma", "qkt_dma", "spread_t", "st_direct", "xsl2", "vdrain_pool", "dbuf2"},
        # 32: restructured batch body: software-pipelined phase A (projections
        # one chunk behind transposes), qn/kn stored full so col-S runs as an
        # A-end block on only 2 PSUM slots, Qt/Kt XBAR transposes on the sync
        # queue (no WAR head-of-line block at batch boundaries), Vt/AcolT on
        # scalar at A-end, x/y DMA on gpsimd.
        32: {"v32"},
        33: {"v32", "xf32r"},
        34: {"v32", "wdirect"},
        35: {"v32", "wdirect", "xf32r"},
        # 36: v32 structure but the proven DVE stt merge (no ACT scaled copy,
        # no Pool add) — isolates the HW divergence seen in 32/35.
        36: {"v32", "dve_stt"},
        37: {"v32", "dve_stt", "wdirect", "xf32r"},
        # bisection of the v32-on-HW corruption:
        38: {"v32", "dve_stt", "prow2"},           # restore ptg double-buffer
        39: {"v32", "dve_stt", "nodbuf2"},         # drop Vn/Vt double-buffer
        40: {"v32", "dve_stt", "tps_sep"},         # v30-style separate transposes
        41: {"v32", "dve_stt", "prow2", "nodbuf2", "tps_sep"},
        # bisect from the v30 side: 42 = v30-equivalent inside the v32 body
        42: {"v32", "dve_stt", "prow2", "nodbuf2", "tps_sep", "cols_inline",
             "octmp_dve", "seq_a", "kt_scalar"},
        43: {"v32", "dve_stt", "prow2", "nodbuf2", "tps_sep", "cols_inline",
             "octmp_dve", "kt_scalar"},                       # +pipelined A
        44: {"v32", "dve_stt", "prow2", "nodbuf2", "tps_sep", "cols_inline",
             "octmp_dve", "seq_a"},                           # +Kt on sync
        45: {"v32", "dve_stt", "prow2", "nodbuf2", "tps_sep",
             "octmp_dve", "seq_a", "kt_scalar"},              # +colS-late+defer
        46: {"v32", "dve_stt", "prow2", "nodbuf2", "tps_sep", "cols_inline",
             "seq_a", "kt_scalar"},                           # +octmp on ACT
    }
    feat = FEAT[variant]
    if (nb, variant, reps) in _NC_CACHE:
        return _NC_CACHE[(nb, variant, reps)]

    import concourse.bass as bass
    import concourse.tile as tile
    from concourse import bacc, mybir
    from concourse.masks import make_identity

    f32 = mybir.dt.float32
    f32r = mybir.dt.float32r
    bf16 = mybir.dt.bfloat16
    AF = mybir.ActivationFunctionType
    AX = mybir.AxisListType

    nc = bacc.Bacc("TRN2")
    xdt = f32r if "xf32r" in feat else f32
    wdt = f32r if "wdirect" in feat else (bf16 if "wb16d" in feat else f32)
    x = nc.dram_tensor("x", [nb, N, E], xdt, kind="ExternalInput")
    w = nc.dram_tensor("w", [E, 3 * E], wdt, kind="ExternalInput")
    bvec = nc.dram_tensor("b", [3 * E], f32, kind="ExternalInput")
    y = nc.dram_tensor("y", [nb, N, E], f32, kind="ExternalOutput")

    with tile.TileContext(nc) as tc:
        with (
            tc.tile_pool(name="const", bufs=1) as constp,
            tc.tile_pool(name="xn", bufs=2 if ("prow3" in feat or "slice_xt" in feat) else 3) as xnp,
            tc.tile_pool(name="big", bufs=1) as bigp,
            tc.tile_pool(name="xsl", bufs=2) as xslp,
            tc.tile_pool(name="qkn", bufs=8 if "st_direct" in feat else 3) as qknp,
            tc.tile_pool(name="prow", bufs=3 if "prow3" in feat else (2 if ("prow2" in feat or "v32" not in feat) else 1)) as prowp,
            tc.tile_pool(name="stat", bufs=3) as statp,
            tc.tile_pool(name="outp", bufs=2) as outpp,
            tc.tile_pool(name="ps_proj", bufs=2, space="PSUM") as ps_proj,
            tc.tile_pool(name="ps_sc", bufs=4, space="PSUM") as ps_sc,
            tc.tile_pool(name="ps_s", bufs=2, space="PSUM") as ps_s,
        ):
            # ---------------- constants ----------------
            # W lands as float32r (rounded by the DVE copy) so fp32r matmuls
            # accept it; staged through the small xn pool to save SBUF.
            # Under wb16 it lands as bf16 instead (full-rate PE matmuls).
            W = constp.tile([128, ED, 3 * E], bf16 if "wb16" in feat else f32r)
            wv = w[:].rearrange("(k p) m -> p k m", p=128)
            if "wdirect" in feat or "wb16d" in feat:
                # dtype matches the dram tensor: DMA W straight in, no
                # staging copies.
                for k in range(ED):
                    nc.sync.dma_start(W[:, k, :], wv[:, k, :])
            else:
                for k in range(ED):
                    for c in range(3):
                        wst = xnp.tile([128, E], f32, tag="xn", name=f"wst{k}_{c}")
                        nc.sync.dma_start(wst, wv[:, k, c * E : (c + 1) * E])
                        nc.vector.tensor_copy(W[:, k, c * E : (c + 1) * E], wst)

            # bias broadcast across partitions (for [n-part, e] layouts)
            b3 = bvec[:].rearrange("(c m) -> c m", m=E)
            bb = constp.tile([128, 3, E], bf16)
            nc.gpsimd.dma_start(
                bb, bass.AP(tensor=b3.tensor, offset=b3.offset, ap=[[0, 128]] + list(b3.ap))
            )
            # bias per partition (for [e-part, n] layouts): bpart[p, c] = b[c*128+p]
            bpart = constp.tile([128, 3 * ED], f32)
            nc.gpsimd.dma_start(bpart, bvec[:].rearrange("(c p) -> p c", p=128))

            ident = constp.tile([128, 128], f32)
            make_identity(nc, ident)
            identB = constp.tile([128, 128], bf16)
            make_identity(nc, identB)
            ones = constp.tile([128, 1], bf16)
            nc.gpsimd.memset(ones, 1.0)
            ones64 = constp.tile([128, 64], bf16)
            nc.gpsimd.memset(ones64, 1.0)
            ones128 = constp.tile([128, 128], bf16)
            nc.gpsimd.memset(ones128, 1.0)
            if "xf32r" in feat:
                # gpsimd memset can't target f32r (invalid ISA) -> copy from
                # the f32 identity instead.
                identX = constp.tile([128, 128], f32r)
                nc.vector.tensor_copy(identX, ident)
            else:
                identX = ident

            def batch_body_v32():
              for b in range(nb):
                Qt = bigp.tile([128, ED, N], bf16, tag="Qt")
                Kt = bigp.tile([128, ED, N], bf16, tag="Kt")
                qf = bigp.tile([128, NE, E], bf16, tag="qf")
                kf = bigp.tile([128, NE, E], bf16, tag="kf")
                # Vn/Vt double-buffered: their writers would otherwise WAR
                # against the previous batch's B2 reads and head-of-line-block
                # the DVE / transpose queues right at the batch boundary.
                vbufs = 1 if "nodbuf2" in feat else 2
                Vn = bigp.tile([128, NE, E], bf16, tag="Vn", bufs=vbufs)
                Vt = bigp.tile([128, ED, N], bf16, tag="Vt", bufs=vbufs)
                Acol = bigp.tile([128, ED, E], bf16, tag="Acol")
                AcolT = bigp.tile([128, ED, E], bf16, tag="AcolT")

                # ---- phase A: chunk pipeline (projections one chunk behind
                # the PE transposes so the PSUM->SBUF copy is off the PE path)
                def emit_load(j):
                    xn = xnp.tile([128, E], xdt, tag="xn")
                    nc.gpsimd.dma_start(xn, x[b, j * 128 : (j + 1) * 128, :])
                    xsl = xslp.tile([128, ED, 128], f32r, tag="xsl", name=f"xsl{b}_{j}")
                    if "tps_sep" in feat:
                        for k in range(ED):
                            tps = ps_proj.tile([128, 128], xdt, tag="ps", name=f"tps{b}_{j}_{k}")
                            nc.tensor.transpose(
                                tps, xn[:, k * 128 : (k + 1) * 128], identX
                            )
                            nc.vector.tensor_copy(xsl[:, k, :], tps)
                    else:
                        tpsb = ps_proj.tile([128, E], xdt, tag="ps")
                        for k in range(ED):
                            nc.tensor.transpose(
                                tpsb[:, k * 128 : (k + 1) * 128],
                                xn[:, k * 128 : (k + 1) * 128],
                                identX,
                            )
                        nc.vector.tensor_copy(xsl.rearrange("p a b -> p (a b)"), tpsb)
                    return xsl

                def emit_proj(j, xsl):
                    jsl = slice(j * 128, (j + 1) * 128)
                    for dst, ci, deng in (
                        (qf[:, j, :], 0, nc.vector),
                        (kf[:, j, :], 1, nc.vector),
                        (Vn[:, j, :], 2, nc.vector),
                    ):
                        pp = ps_proj.tile([128, E], f32, tag="ps")
                        for k in range(ED):
                            nc.tensor.matmul(
                                pp,
                                xsl[:, k, :],
                                W[:, k, ci * E : (ci + 1) * E],
                                start=(k == 0),
                                stop=(k == ED - 1),
                            )
                        deng.tensor_add(dst, pp, bb[:, ci, :])
                    nc.sync.dma_start_transpose(Qt[:, :, jsl], qf[:, j, :])
                    kteng = nc.scalar if "kt_scalar" in feat else nc.sync
                    kteng.dma_start_transpose(Kt[:, :, jsl], kf[:, j, :])
                    vteng = (
                        (nc.sync if j % 2 else nc.scalar)
                        if "kt_scalar" in feat
                        else nc.scalar
                    )
                    vteng.dma_start_transpose(Vt[:, :, jsl], Vn[:, j, :])

                if "seq_a" in feat:
                    for j in range(NE):
                        emit_proj(j, emit_load(j))
                else:
                    prev = None
                    for j in range(NE):
                        xsl_j = emit_load(j)
                        if prev is not None:
                            emit_proj(*prev)
                        prev = (j, xsl_j)
                    emit_proj(*prev)

                # ---- col-S as an A-end block: 2 PSUM slots at a time, so the
                # previous batch's B2 po/oc can keep cycling the other 2.
                cstat = statp.tile([128, 3 * ED], f32, tag="cstat")

                def emit_colS_half(half):
                    cs = [
                        ps_sc.tile([128, E], f32, tag="scps", name=f"cs{b}_{half}_{i2}")
                        for i2 in range(2)
                    ]
                    for c in range(NE):
                        for i2 in range(2):
                            i = half * 2 + i2
                            nc.tensor.matmul(
                                cs[i2],
                                qf[:, c, i * 128 : (i + 1) * 128],
                                kf[:, c, :],
                                start=(c == 0),
                                stop=(c == NE - 1),
                            )
                    return cs

                def emit_colsm_half(half, cs):
                    for i2 in range(2):
                        i = half * 2 + i2
                        nm = cstat[:, 2 * ED + i : 2 * ED + i + 1]
                        nc.vector.reduce_max(nm, cs[i2], axis=AX.X, negate=True)
                        nc.vector.tensor_scalar_mul(nm, nm, SCALE)
                        nc.scalar.activation(
                            out=Acol[:, i, :],
                            in_=cs[i2],
                            func=AF.Exp,
                            scale=SCALE,
                            bias=nm,
                            accum_out=cstat[:, i : i + 1],
                        )

                def emit_colsm_finish():
                    nc.vector.reciprocal(cstat[:, ED : 2 * ED], cstat[:, 0:ED])
                    for i in range(ED):
                        nc.vector.tensor_scalar_mul(
                            Acol[:, i, :], Acol[:, i, :], cstat[:, ED + i : ED + i + 1]
                        )
                        nc.scalar.dma_start_transpose(
                            AcolT[:, :, i * 128 : (i + 1) * 128], Acol[:, i, :]
                        )

                if "cols_inline" in feat:
                    csall = [
                        ps_sc.tile([128, E], f32, tag="scps", name=f"cs{b}_{i}")
                        for i in range(ED)
                    ]
                    for c in range(NE):
                        for i in range(ED):
                            nc.tensor.matmul(
                                csall[i],
                                qf[:, c, i * 128 : (i + 1) * 128],
                                kf[:, c, :],
                                start=(c == 0),
                                stop=(c == NE - 1),
                            )
                    emit_colsm_half(0, csall[0:2])
                    emit_colsm_half(1, csall[2:4])
                    emit_colsm_finish()
                    cs1 = None
                else:
                    cs0 = emit_colS_half(0)
                    emit_colsm_half(0, cs0)
                    cs1 = emit_colS_half(1)
                # A2 for the second half is deferred into B1 g0 (below) so the
                # first St exps aren't queued behind the col-softmax on ACT.

                # ---- phase B: identical structure to st_direct
                for g in range(4):
                    gsl = slice(g * 512, (g + 1) * 512)
                    ptg = prowp.tile([128, NE, 512], bf16, tag="ptg")
                    for m in range(NE):
                        if g == 0 and m == 4 and cs1 is not None:
                            # deferred col-softmax half 1: its ACT exps queue
                            # behind B1's first St exps instead of ahead.
                            emit_colsm_half(1, cs1)
                            emit_colsm_finish()
                        sps = ps_s.tile([128, 512], f32, tag="s")
                        for k in range(ED):
                            nc.tensor.matmul(
                                sps,
                                Kt[:, k, m * 128 : (m + 1) * 128],
                                Qt[:, k, gsl],
                                start=(k == 0),
                                stop=(k == ED - 1),
                            )
                        nc.scalar.activation(
                            out=ptg[:, m, :], in_=sps, func=AF.Exp, scale=SCALE
                        )
                    ocs = {}
                    def emit_oc(j):
                        oc = ps_sc.tile([128, E], f32, tag="scps", name=f"oc{b}_{j}")
                        for c in range(ED):
                            nc.tensor.matmul(
                                oc,
                                Vt[:, c, j * 128 : (j + 1) * 128],
                                AcolT[:, c, :],
                                start=(c == 0),
                                stop=(c == ED - 1),
                            )
                        ocs[j] = oc
                    emit_oc(g * 4)
                    emit_oc(g * 4 + 1)
                    for jj in range(4):
                        j = g * 4 + jj
                        jpart = slice(jj * 128, (jj + 1) * 128)
                        po = ps_sc.tile([128, E], f32, tag="scps")
                        rs = ps_s.tile([128, 8], f32, tag="s", name=f"rs{b}_{j}")
                        for m in range(NE):
                            nc.tensor.matmul(
                                po,
                                ptg[:, m, jpart],
                                Vn[:, m, :],
                                start=(m == 0),
                                stop=(m == NE - 1),
                            )
                            nc.tensor.matmul(
                                rs[:, 0:1],
                                ptg[:, m, jpart],
                                ones,
                                start=(m == 0),
                                stop=(m == NE - 1),
                            )
                        rstat = statp.tile([128, 8], f32, tag="rstat")
                        nc.vector.reciprocal(rstat[:, 5:6], rs[:, 0:1])
                        if jj < 2:
                            emit_oc(g * 4 + 2 + jj)
                        ot = outpp.tile([128, E], f32, tag="ot")
                        octmp = outpp.tile([128, E], f32, tag="octmp")
                        if "dve_stt" in feat:
                            if "octmp_dve" in feat:
                                nc.vector.tensor_copy(octmp, ocs.pop(j))
                            else:
                                nc.scalar.activation(
                                    out=octmp, in_=ocs.pop(j), func=AF.Copy
                                )
                            nc.vector.scalar_tensor_tensor(
                                ot,
                                po,
                                rstat[:, 5:6],
                                octmp,
                                op0=mybir.AluOpType.mult,
                                op1=mybir.AluOpType.add,
                            )
                        else:
                            potmp = outpp.tile([128, E], f32, tag="potmp")
                            # merge off DVE entirely: ACT drains oc and the
                            # 1/rowsum-scaled po (per-partition scale), Pool
                            # does the SBUF-only add.
                            nc.scalar.activation(
                                out=octmp, in_=ocs.pop(j), func=AF.Copy
                            )
                            nc.scalar.activation(
                                out=potmp, in_=po, func=AF.Copy, scale=rstat[:, 5:6]
                            )
                            nc.gpsimd.tensor_add(ot, potmp, octmp)
                        nc.gpsimd.dma_start(y[b, j * 128 : (j + 1) * 128, :], ot)

            def batch_body():
              if "v32" in feat:
                  return batch_body_v32()
              for b in range(nb):
                tpeng = nc.gpsimd if "tp_gpsimd" in feat else nc.scalar
                qkt_dt = bf16 if "qkt_dma" in feat else f32r
                vn_dt = f32r if "f32r_pv" in feat else bf16
                dbufs = 2 if "dbuf" in feat else 1
                dbufs2 = 2 if "dbuf2" in feat else dbufs
                slice_xt = "slice_xt" in feat or "xsl2" in feat
                if not slice_xt and "wb16" not in feat:
                    xT = bigp.tile([128, ED, N], f32r, tag="xT")
                if "qkv1t" in feat:
                    QKVt = bigp.tile([128, 3 * ED, N], qkt_dt, tag="QKVt")
                    QtA = lambda k, sl: QKVt[:, k, sl]
                    KtA = lambda k, sl: QKVt[:, ED + k, sl]
                    VtA = lambda c, sl: QKVt[:, 2 * ED + c, sl]
                elif "qk1t" in feat:
                    QKt = bigp.tile([128, 2 * ED, N], qkt_dt, tag="QKt")
                    QtA = lambda k, sl: QKt[:, k, sl]
                    KtA = lambda k, sl: QKt[:, ED + k, sl]
                else:
                    Qt = bigp.tile([128, ED, N], qkt_dt, tag="Qt")
                    Kt = bigp.tile([128, ED, N], qkt_dt, tag="Kt")
                    QtA = lambda k, sl: Qt[:, k, sl]
                    KtA = lambda k, sl: Kt[:, k, sl]
                Vn = bigp.tile([128, NE, E], vn_dt, tag="Vn", bufs=dbufs2)
                if "qkv1t" not in feat:
                    Vt = bigp.tile([128, ED, N], bf16, tag="Vt")
                    VtA = lambda c, sl: Vt[:, c, sl]
                Acol = bigp.tile([128, ED, E], bf16, tag="Acol", bufs=dbufs)
                AcolT = bigp.tile([128, ED, E], bf16, tag="AcolT", bufs=dbufs2)
                scol_ps = [
                    ps_sc.tile([128, E], f32, tag="scps", name=f"scol{b}_{i}")
                    for i in range(ED)
                ]

                # ---- phase A: load x, build xT, projections, col-S accumulation
                for j in range(NE):
                    s_idx, jj = j // ED, j % ED
                    if "wb16" in feat:
                        xT = xslp.tile(
                            [128, ED, 128], bf16, tag="xsl", name=f"xsl{b}_{j}"
                        )
                        xoff, roff = 0, 0
                    elif "xsl2" in feat:
                        xT = xslp.tile(
                            [128, ED, 128], f32r, tag="xsl", name=f"xsl{b}_{j}"
                        )
                        xoff, roff = 0, 0
                    elif slice_xt:
                        if jj == 0:
                            xT = xslp.tile(
                                [128, ED, ED * 128], f32r, tag="xsl", name=f"xsl{b}_{s_idx}"
                            )
                        xoff, roff = jj * 128, 0
                    else:
                        xoff, roff = j * 128, s_idx * 512
                    xn = xnp.tile([128, E], f32, tag="xn")
                    xeng = (
                        nc.gpsimd
                        if ("spread_t" in feat or "spread_t2" in feat)
                        else nc.sync
                    )
                    xeng.dma_start(xn, x[b, j * 128 : (j + 1) * 128, :])
                    if "xb16t" in feat:
                        # cast x to bf16 first so the PE transposes run at
                        # bf16 rate (f32 transposes are 2x slower)
                        xnB = qknp.tile([128, E], bf16, tag="xnB", name=f"xnB{b}_{j}")
                        nc.vector.tensor_copy(xnB, xn)
                        tpsb = ps_proj.tile([128, E], bf16, tag="ps")
                        for k in range(ED):
                            nc.tensor.transpose(
                                tpsb[:, k * 128 : (k + 1) * 128],
                                xnB[:, k * 128 : (k + 1) * 128],
                                identB,
                            )
                        nc.vector.tensor_copy(
                            xT.rearrange("p a b -> p (a b)"), tpsb
                        )
                    elif "xsl2" in feat or "wb16" in feat:
                        # all 4 transposes land in one PSUM tile -> one copy
                        # (under wb16 the copy also casts f32 -> bf16)
                        tpsb = ps_proj.tile([128, E], f32, tag="ps")
                        for k in range(ED):
                            nc.tensor.transpose(
                                tpsb[:, k * 128 : (k + 1) * 128],
                                xn[:, k * 128 : (k + 1) * 128],
                                ident,
                            )
                        nc.vector.tensor_copy(
                            xT.rearrange("p a b -> p (a b)"), tpsb
                        )
                    else:
                        for k in range(ED):
                            tps = ps_proj.tile([128, 128], f32, tag="ps")
                            nc.tensor.transpose(tps, xn[:, k * 128 : (k + 1) * 128], ident)
                            nc.vector.tensor_copy(xT[:, k, xoff : xoff + 128], tps)

                    # natural-layout q, k, v for this token chunk
                    if "qkv1t" in feat:
                        qkn = qknp.tile([128, 3 * E], bf16, tag="qkvn")
                        qn = qkn[:, 0:E]
                        kn = qkn[:, E : 2 * E]
                    elif "qk1t" in feat:
                        qkn = qknp.tile([128, 2 * E], bf16, tag="qkn")
                        qn = qkn[:, 0:E]
                        kn = qkn[:, E : 2 * E]
                    else:
                        qn = qknp.tile([128, E], bf16, tag="qn")
                        kn = qknp.tile([128, E], bf16, tag="kn")
                    for dst, ci in ((qn, 0), (kn, 1), (Vn[:, j, :], 2)):
                        pp = ps_proj.tile([128, E], f32, tag="ps")
                        for k in range(ED):
                            nc.tensor.matmul(
                                pp,
                                xT[:, k, xoff : xoff + 128],
                                W[:, k, ci * E : (ci + 1) * E],
                                start=(k == 0),
                                stop=(k == ED - 1),
                            )
                        deng = (
                            nc.gpsimd
                            if (ci == 2 and "vdrain_pool" in feat)
                            else nc.vector
                        )
                        deng.tensor_add(dst, pp, bb[:, ci, :])
                        if ci == 2 and "qkv1t" in feat:
                            # duplicate v into the contiguous qkv tile so one
                            # XBAR call transposes q|k|v together
                            nc.vector.tensor_copy(
                                qkn[:, 2 * E : 3 * E], Vn[:, j, :]
                            )

                    # col-attention S accumulation: S_col[d,e] += q_j.T @ k_j
                    for i in range(ED):
                        nc.tensor.matmul(
                            scol_ps[i],
                            qn[:, i * 128 : (i + 1) * 128],
                            kn,
                            start=(j == 0),
                            stop=(j == NE - 1),
                        )

                    # bf16 transposed layouts via the DMA XBAR (free wrt PE)
                    jsl = slice(j * 128, (j + 1) * 128)
                    if "pe_vt" in feat:
                        for kq in range(ED):
                            vps = ps_proj.tile(
                                [128, 128], bf16, tag="ps", name=f"vps{b}_{j}_{kq}"
                            )
                            nc.tensor.transpose(
                                vps, Vn[:, j, kq * 128 : (kq + 1) * 128], identB
                            )
                            nc.vector.tensor_copy(Vt[:, kq, jsl], vps)
                    if "qkt_dma" in feat and "vt_late" in feat:
                        # B1 consumes Kt/Qt first: put them at the queue head
                        nc.scalar.dma_start_transpose(Qt[:, :, jsl], qn)
                        nc.scalar.dma_start_transpose(Kt[:, :, jsl], kn)
                    if "vt_dma" in feat and "vt_late" not in feat:
                        veng = (
                            (nc.sync if j % 2 else nc.scalar)
                            if "spread_t" in feat
                            else tpeng
                        )
                        veng.dma_start_transpose(Vt[:, :, jsl], Vn[:, j, :])
                    if "qkt_dma" in feat and "vt_late" not in feat:
                        if "qkv1t" in feat:
                            tpeng.dma_start_transpose(QKVt[:, :, jsl], qkn)
                        elif "qk1t" in feat:
                            tpeng.dma_start_transpose(QKt[:, :, jsl], qkn)
                        else:
                            qeng = nc.sync if "spread_t" in feat else tpeng
                            qeng.dma_start_transpose(Qt[:, :, jsl], qn)
                            tpeng.dma_start_transpose(Kt[:, :, jsl], kn)

                    # transposed-layout projections, one 512-token slice at a time
                    if "qkt_dma" in feat:
                        tproj = ()
                    elif "vt_dma" in feat:
                        tproj = ((Qt, 0), (Kt, 1))
                    else:
                        tproj = ((Qt, 0), (Kt, 1), (Vt, 2))
                    if j % ED == ED - 1 and tproj:
                        sl = slice(s_idx * 512, (s_idx + 1) * 512)
                        for dst, ci in tproj:
                            for i in range(ED):
                                pp = ps_proj.tile([128, E], f32, tag="ps")
                                for k in range(ED):
                                    nc.tensor.matmul(
                                        pp,
                                        W[:, k, ci * E + i * 128 : ci * E + (i + 1) * 128],
                                        xT[:, k, roff : roff + 512],
                                        start=(k == 0),
                                        stop=(k == ED - 1),
                                    )
                                if "act_drain" in feat:
                                    nc.scalar.activation(
                                        out=dst[:, i, sl],
                                        in_=pp,
                                        func=AF.Identity,
                                        bias=bpart[:, ci * ED + i : ci * ED + i + 1],
                                    )
                                else:
                                    nc.vector.tensor_scalar_add(
                                        dst[:, i, sl], pp, bpart[:, ci * ED + i : ci * ED + i + 1]
                                    )

                # ---- phase A2: col softmax + transpose of A
                # col logits are O(+-600): subtract the per-row max (as an ACT
                # bias of -max*SCALE) before exp, unlike the row path.
                cstat = statp.tile([128, 3 * ED], f32, tag="cstat")
                for i in range(ED):
                    if "col_nomax" in feat:
                        # logits*SCALE are O(+-27) (empirically +-600 raw):
                        # exp fits f32 with 3x margin, skip the max chain
                        nc.scalar.activation(
                            out=Acol[:, i, :],
                            in_=scol_ps[i],
                            func=AF.Exp,
                            scale=SCALE,
                            accum_out=cstat[:, i : i + 1],
                        )
                        continue
                    nm = cstat[:, 2 * ED + i : 2 * ED + i + 1]
                    nc.vector.reduce_max(nm, scol_ps[i], axis=AX.X, negate=True)
                    nc.vector.tensor_scalar_mul(nm, nm, SCALE)
                    nc.scalar.activation(
                        out=Acol[:, i, :],
                        in_=scol_ps[i],
                        func=AF.Exp,
                        scale=SCALE,
                        bias=nm,
                        accum_out=cstat[:, i : i + 1],
                    )
                nc.vector.reciprocal(cstat[:, ED : 2 * ED], cstat[:, 0:ED])
                for i in range(ED):
                    nc.vector.tensor_scalar_mul(
                        Acol[:, i, :], Acol[:, i, :], cstat[:, ED + i : ED + i + 1]
                    )
                    tpeng.dma_start_transpose(
                        AcolT[:, :, i * 128 : (i + 1) * 128], Acol[:, i, :]
                    )

                if "vt_late" in feat:
                    for jv in range(NE):
                        nc.scalar.dma_start_transpose(
                            Vt[:, :, jv * 128 : (jv + 1) * 128], Vn[:, jv, :]
                        )

                # ---- phase B (st_direct): S^T = K Q^T per 512-token j-group;
                # exp() output IS P~^T (no transposes); row sums via a ones
                # column reusing the PV stationary; 1/rowsum at the merge.
                if "st_direct" in feat:
                    for g in range(4):
                        gsl = slice(g * 512, (g + 1) * 512)
                        ptg = prowp.tile([128, NE, 512], bf16, tag="ptg")
                        if "rs_group" in feat:
                            rsall = ps_proj.tile(
                                [128, 512], f32, tag="ps", name=f"rsall{b}_{g}"
                            )
                        for m in range(NE):
                            sps = ps_s.tile([128, 512], f32, tag="s")
                            for k in range(ED):
                                nc.tensor.matmul(
                                    sps,
                                    KtA(k, slice(m * 128, (m + 1) * 128)),
                                    QtA(k, gsl),
                                    start=(k == 0),
                                    stop=(k == ED - 1),
                                )
                            nc.scalar.activation(
                                out=ptg[:, m, :], in_=sps, func=AF.Exp, scale=SCALE
                            )
                        if "rs_group" in feat:
                            # replicated rowsums, emitted as a block AFTER the
                            # St/exp loop: each matmul depends on exp(m), and
                            # emitting it inline would serialize the PE queue
                            # behind the ACT chain at every m-step.
                            for m in range(NE):
                                nc.tensor.matmul(
                                    rsall,
                                    ones128,
                                    ptg[:, m, :],
                                    start=(m == 0),
                                    stop=(m == NE - 1),
                                )
                        # col-attention output for this group's 4 chunks: fills
                        # the PE while the last exp() quarters drain.
                        ocs = {}
                        def emit_oc(j):
                            oc = ps_sc.tile(
                                [128, E], f32, tag="scps", name=f"oc{b}_{j}"
                            )
                            for c in range(ED):
                                nc.tensor.matmul(
                                    oc,
                                    VtA(c, slice(j * 128, (j + 1) * 128)),
                                    AcolT[:, c, :],
                                    start=(c == 0),
                                    stop=(c == ED - 1),
                                )
                            ocs[j] = oc
                        emit_oc(g * 4)
                        emit_oc(g * 4 + 1)
                        for jj in range(4):
                            j = g * 4 + jj
                            jpart = slice(jj * 128, (jj + 1) * 128)
                            po = ps_sc.tile([128, E], f32, tag="scps")
                            if "rs_group" not in feat:
                                rs = (ps_proj if "rs_proj" in feat else ps_s).tile(
                                    [128, 64 if "ones64" in feat else 8], f32,
                                    tag="ps" if "rs_proj" in feat else "s",
                                    name=f"rs{b}_{j}",
                                )
                                onesv = ones64 if "ones64" in feat else ones
                                rsl = slice(0, 64) if "ones64" in feat else slice(0, 1)
                            for m in range(NE):
                                nc.tensor.matmul(
                                    po,
                                    ptg[:, m, jpart],
                                    Vn[:, m, :],
                                    start=(m == 0),
                                    stop=(m == NE - 1),
                                )
                                if "rs_group" not in feat:
                                    nc.tensor.matmul(
                                        rs[:, rsl],
                                        ptg[:, m, jpart],
                                        onesv,
                                        start=(m == 0),
                                        stop=(m == NE - 1),
                                    )
                            rstat = statp.tile([128, 8], f32, tag="rstat")
                            if "rs_group" in feat:
                                # diagonal pick: accum = sum_q(rsall*I) per
                                # partition = this chunk's rowsums
                                junk = statp.tile([128, 128], f32, tag="junk")
                                nc.vector.scalar_tensor_tensor(
                                    out=junk,
                                    in0=rsall[:, jpart],
                                    scalar=1.0,
                                    in1=ident,
                                    op0=mybir.AluOpType.mult,
                                    op1=mybir.AluOpType.mult,
                                    accum_out=rstat[:, 4:5],
                                )
                                nc.vector.reciprocal(rstat[:, 5:6], rstat[:, 4:5])
                            else:
                                nc.vector.reciprocal(rstat[:, 5:6], rs[:, 0:1])
                            if jj < 2:
                                emit_oc(g * 4 + 2 + jj)
                            ot = outpp.tile([128, E], f32, tag="ot")
                            octmp = outpp.tile([128, E], f32, tag="octmp")
                            nc.vector.tensor_copy(octmp, ocs.pop(j))
                            nc.vector.scalar_tensor_tensor(
                                ot,
                                po,
                                rstat[:, 5:6],
                                octmp,
                                op0=mybir.AluOpType.mult,
                                op1=mybir.AluOpType.add,
                            )
                            yeng2 = (
                                nc.gpsimd
                                if ("spread_t" in feat or "spread_t2" in feat)
                                else nc.sync
                            )
                            yeng2.dma_start(
                                y[b, j * 128 : (j + 1) * 128, :], ot
                            )
                    if "bbar" in feat and b < nb - 1:
                        # empty 1-iteration hardware loop = supported
                        # all-engine barrier between batches
                        with tc.For_i(0, 1, 1):
                            pass
                    continue

                # ---- phase B: row attention + merged output, per token chunk
                early_t = "early_t" in feat
                late_norm = "late_norm" in feat
                spread = "spread" in feat
                for j in range(NE):
                    teng = (nc.sync if j % 2 else nc.scalar) if spread else nc.scalar
                    yeng = (nc.scalar if j % 2 else nc.sync) if spread else nc.sync
                    if "spread_t" in feat:
                        yeng = nc.gpsimd
                    pt = prowp.tile([128, N], bf16, tag="pt")
                    ptT = prowp.tile(
                        [128, NE, 128], f32r if "f32r_pv" in feat else bf16, tag="ptT"
                    )
                    rstat = statp.tile([128, 8], f32, tag="rstat")
                    for q in range(4):
                        sps = ps_s.tile([128, 512], f32, tag="s")
                        for k in range(ED):
                            nc.tensor.matmul(
                                sps,
                                Qt[:, k, j * 128 : (j + 1) * 128],
                                Kt[:, k, q * 512 : (q + 1) * 512],
                                start=(k == 0),
                                stop=(k == ED - 1),
                            )
                        nc.scalar.activation(
                            out=pt[:, q * 512 : (q + 1) * 512],
                            in_=sps,
                            func=AF.Exp,
                            scale=SCALE,
                            accum_out=rstat[:, q : q + 1],
                        )
                        if early_t:
                            # transpose the unnormalized quarter right away;
                            # 1/rowsum is applied to the PV output instead
                            teng.dma_start_transpose(
                                ptT[:, 4 * q : 4 * q + 4, :],
                                pt[:, q * 512 : (q + 1) * 512],
                            )
                        if "pe_pt" in feat:
                            for t in range(4):
                                m = 4 * q + t
                                psB = ps_proj.tile(
                                    [128, 128], bf16, tag="ps", name=f"psB{b}_{j}_{m}"
                                )
                                nc.tensor.transpose(
                                    psB, pt[:, m * 128 : (m + 1) * 128], identB
                                )
                                nc.vector.tensor_copy(ptT[:, m, :], psB)
                    nc.vector.reduce_sum(rstat[:, 4:5], rstat[:, 0:4], axis=AX.X)
                    nc.vector.reciprocal(rstat[:, 5:6], rstat[:, 4:5])
                    if late_norm:
                        if "pe_pt" not in feat:
                            teng.dma_start_transpose(ptT, pt)
                    elif not early_t:
                        nc.vector.tensor_scalar_mul(pt, pt, rstat[:, 5:6])
                        if "fake_t" in feat:
                            teng.dma_start(ptT.rearrange("p a b -> p (a b)"), pt)
                        elif "no_t" in feat:
                            nc.vector.tensor_copy(ptT[:, 0, :], pt[:, :128])
                        else:
                            teng.dma_start_transpose(ptT, pt)

                    po = ps_sc.tile([128, E], f32, tag="scps")
                    for m in range(NE):
                        nc.tensor.matmul(
                            po,
                            ptT[:, m, :],
                            Vn[:, m, :],
                            start=(m == 0),
                            stop=((early_t or late_norm) and m == NE - 1),
                        )
                    ot = outpp.tile([128, E], f32, tag="ot")
                    if early_t or late_norm:
                        oc = ps_sc.tile([128, E], f32, tag="scps")
                        for c in range(ED):
                            nc.tensor.matmul(
                                oc,
                                Vt[:, c, j * 128 : (j + 1) * 128],
                                AcolT[:, c, :],
                                start=(c == 0),
                                stop=(c == ED - 1),
                            )
                        if "dve_merge" in feat:
                            if "psum2" in feat:
                                nc.vector.scalar_tensor_tensor(
                                    ot,
                                    po,
                                    rstat[:, 5:6],
                                    oc,
                                    op0=mybir.AluOpType.mult,
                                    op1=mybir.AluOpType.add,
                                )
                            else:
                                octmp = outpp.tile([128, E], f32, tag="octmp")
                                nc.vector.tensor_copy(octmp, oc)
                                nc.vector.scalar_tensor_tensor(
                                    ot,
                                    po,
                                    rstat[:, 5:6],
                                    octmp,
                                    op0=mybir.AluOpType.mult,
                                    op1=mybir.AluOpType.add,
                                )
                        else:
                            nc.scalar.activation(
                                out=ot, in_=po, func=AF.Copy, scale=rstat[:, 5:6]
                            )
                            nc.vector.tensor_add(ot, ot, oc)
                    else:
                        for c in range(ED):
                            nc.tensor.matmul(
                                po,
                                Vt[:, c, j * 128 : (j + 1) * 128],
                                AcolT[:, c, :],
                                start=False,
                                stop=(c == ED - 1),
                            )
                        nc.vector.tensor_copy(ot, po)
                    yeng.dma_start(y[b, j * 128 : (j + 1) * 128, :], ot)

            if reps == 1:
                batch_body()
            else:
                with tc.For_i(0, reps, 1):
                    batch_body()

    nc.compile()
    _NC_CACHE[(nb, variant, reps)] = nc
    return nc


def make_in_maps(x, w_qkv, b_qkv):
    xs = np.ascontiguousarray(np.asarray(x, dtype=np.float32)).reshape(B, N, E)
    w = np.ascontiguousarray(np.asarray(w_qkv, dtype=np.float32))
    bq = np.ascontiguousarray(np.asarray(b_qkv, dtype=np.float32))
    return [
        {"x": np.ascontiguousarray(xs[c * NB : (c + 1) * NB]), "w": w, "b": bq}
        for c in range(NCORES)
    ]


BEST_VARIANT = 69

_DISPATCH_CACHE = {}


def _get_dispatch(variant=BEST_VARIANT):
    """Build (once) the persistent jitted 8-core dispatcher for the kernel.

    Mirrors concourse.bass2jax.run_bass_via_pjrt's shard_map structure (the
    bass_exec custom call only tolerates parameters as operands), but:
      - the jit + device buffers are cached at module level so repeat
        kernel() calls skip retrace/recompile;
      - w/b are replicated via PartitionSpec() instead of 8x host-concat;
      - the ExternalOutput zero-operand is NOT donated: the kernel writes
        every element of y, so one cached device-resident zeros buffer is
        reused across calls (no 128MB host->device zero upload per call).
    """
    if variant in _DISPATCH_CACHE:
        return _DISPATCH_CACHE[variant]

    import jax
    from jax.experimental.shard_map import shard_map
    from jax.sharding import Mesh, NamedSharding, PartitionSpec

    from concourse.bass2jax import (
        _bass_exec_p,
        install_neuronx_cc_hook,
        partition_id_tensor,
    )

    install_neuronx_cc_hook()
    nc = build_nc(NB, variant)
    pname = nc.partition_id_tensor.name if nc.partition_id_tensor else None
    in_names = ("x", "w", "b", "y") + ((pname,) if pname else ())
    out_avals = (jax.core.ShapedArray((NB, N, E), np.float32),)

    from concourse import mybir

    in_dtypes = {}
    for alloc in nc.m.functions[0].allocations:
        if isinstance(alloc, mybir.MemoryLocationSet) and alloc.kind == "ExternalInput":
            in_dtypes[alloc.memorylocations[0].name] = mybir.dt.np(alloc.dtype)

    def _body(x_, w_, b_, z_):
        operands = [x_, w_, b_, z_]
        if pname is not None:
            operands.append(partition_id_tensor())
        outs = _bass_exec_p.bind(
            *operands,
            out_avals=out_avals,
            in_names=in_names,
            out_names=("y",),
            lowering_input_output_aliases=(),
            sim_require_finite=True,
            sim_require_nnan=True,
            nc=nc,
        )
        return outs[0]

    devices = jax.devices()[:NCORES]
    mesh = Mesh(np.asarray(devices), ("core",))
    P = PartitionSpec
    fn = jax.jit(
        shard_map(
            _body,
            mesh=mesh,
            in_specs=(P("core"), P(), P(), P("core")),
            out_specs=P("core"),
            check_rep=False,
        ),
        keep_unused=True,
    )
    shx = NamedSharding(mesh, P("core"))
    shr = NamedSharding(mesh, P())
    zeros = jax.device_put(np.zeros((B, N, E), np.float32), shx)
    d = (nc, fn, shx, shr, zeros, in_dtypes)
    _DISPATCH_CACHE[variant] = d
    return d


def _put_inputs(x, w_qkv, b_qkv, variant):
    import jax

    nc, fn, shx, shr, zeros, in_dtypes = _get_dispatch(variant)
    xs = np.ascontiguousarray(
        np.asarray(x).reshape(B, N, E).astype(in_dtypes["x"], copy=False)
    )
    xd = jax.device_put(xs, shx)
    wd = jax.device_put(
        np.ascontiguousarray(np.asarray(w_qkv).astype(in_dtypes["w"], copy=False)), shr
    )
    bd = jax.device_put(
        np.ascontiguousarray(np.asarray(b_qkv).astype(in_dtypes["b"], copy=False)), shr
    )
    return nc, fn, (xd, wd, bd, zeros)


def kernel(x, w_qkv, b_qkv):
    nc, fn, args = _put_inputs(x, w_qkv, b_qkv, BEST_VARIANT)
    return np.asarray(fn(*args), dtype=np.float32)


def get_dispatcher(x, w_qkv, b_qkv, variant=BEST_VARIANT):
    """For profiling: returns (nc, run_fn) where run_fn() performs exactly
    one on-device dispatch with device-resident inputs and blocks."""
    import jax

    nc, fn, args = _put_inputs(x, w_qkv, b_qkv, variant)

    def run_fn():
        return jax.block_until_ready(fn(*args))

    return nc, run_fn

